# revision 82
# baseline (speedup 1.0000x reference)
"""Trainium2 Bass kernel for nn_EquivariantLayer (spectral equivariant layer).

Strategy (data-parallel over batch, 2 samples/core x 8 cores):
  All FFTs are expressed as real DFT matmuls on the TensorEngine with layouts
  chosen so no corner-turn transposes are ever needed:

    stage1:  A = f^T @ [ExR^T | ExI^T]          (contract x; out [y, (RI,kx)])
    stage2:  F = Ey @ A                          (contract y; out [c, kx], c-major)
    conv:    M = F (*) K elementwise (K = rfft2(sym kernel) is REAL since the
             symmetrized kernel is D4-symmetric); the i-reduction runs as a
             "selector-transpose" matmul (stationary = M-slice, moving =
             selector) so the conv spectrum lands TRANSPOSED [kx, (j, c)]
    uncurl:  TO_U = i*t, TO_V = i*s are pure-imaginary -> one fused real mult
             per field over the partition-stacked [acv_I; acv_R] PSUM tile
    synth:   X-first 2-stage iDFT: psHT[(4j,c),(RI,X)] = BT^T @ PPbig, then
             psF[X,(4j,Y)] = sum_c H_R QYc + H_I QYs with K=64 zero-padded
             stationaries (K=32 stationary matmuls fault on TRN2 HW)
    cross:   u_a v_b - u_b v_a: fused [128,2048] bf16 products (16 pairs per
             DVE op via zero-step broadcast APs), f32 subtract on gpsimd/DVE

  The compute pipeline runs in bf16 (PE at 1 cycle/row, DVE 2x mode); PSUM
  accumulation stays fp32 and the final subtraction materializes fp32 output.
  Output DMAs round-robin over the SP / ACT / SWDGE rings (SP-weighted).
  HW constraints honored: gpsimd never touches PSUM; tensor_tensor operands
  share a partition base; PE stationary bases are 0/32/64 with K >= 64.

Output [16, 128, 128, 128] f32 (~134 MB) dominates traffic (memory regime).
"""
import sys
import numpy as np
import ml_dtypes

if '/opt/trn_rl_repo' not in sys.path:
    sys.path.insert(0, '/opt/trn_rl_repo')

import concourse.bass as bass
from concourse import bacc
import concourse.mybir as mybir
import concourse.tile as tile
from concourse.bass import AP
from concourse.bass_utils import run_bass_kernel_spmd

F32 = mybir.dt.float32
BF16 = mybir.dt.bfloat16
N_CORES = 8
B_PER_CORE = 2
C1, C2, N1, N2 = 8, 16, 64, 128
NCH_OUT = 128  # 8 fr + 120 cross

I_IDX, J_IDX = np.triu_indices(C2, 1)
_PAIR_IDX = {}
for _p, (_a, _b) in enumerate(zip(I_IDX, J_IDX)):
    _PAIR_IDX[(int(_a), int(_b))] = _p


# ---------------------------------------------------------------------------
# host-side constant construction
# ---------------------------------------------------------------------------

def _host_consts():
    x = np.arange(64)
    kx = np.arange(64)
    c = np.arange(32)
    y = np.arange(64)
    X = np.arange(128)
    Y = np.arange(128)

    FRs = np.where(kx <= 32, kx, kx - 64).astype(np.float64)  # signed row freq

    ExR = np.cos(2 * np.pi * np.outer(kx, x) / 64)   # [kx, x]
    ExI = -np.sin(2 * np.pi * np.outer(kx, x) / 64)
    # [A_R | A_I | -A_R] so stage2 fuses R/I into two matmuls
    ExF = np.concatenate([ExR.T, ExI.T, -ExR.T], axis=1)   # [x, 192]

    # F_R = C A_R + S A_I ; F_I = C A_I + S (-A_R)   (C=cos, S=sin)
    # replicated x2 down partitions so base-0 and base-64 slices both exist
    EyCT = np.tile(np.cos(2 * np.pi * np.outer(c, y) / 64).T, (2, 1))  # [128, 32]
    EyST = np.tile(np.sin(2 * np.pi * np.outer(c, y) / 64).T, (2, 1))

    S_sel = np.zeros((128, 32))
    for im in range(4):
        S_sel[im * 32 + np.arange(32), np.arange(32)] = 1.0

    den = FRs[None, :] ** 2 + c[:, None].astype(np.float64) ** 2
    den[0, 0] = 1.0
    t_u = c[:, None] / den                           # [32, 64]
    s_v = -FRs[None, :] / den

    # uncurl consts in transposed [kx, (j-rep 16, c 32)] layout, partition-
    # aligned with psT2 = [acv_I (rows 0-63); acv_R (rows 64-127)]:
    #   BTu = psT2 * [-t; +t],  BTv = psT2 * [-s; +s]
    tmat = np.tile(t_u.T[:, None, :], (1, 16, 1)).reshape(64, 512)
    smat = np.tile(s_v.T[:, None, :], (1, 16, 1)).reshape(64, 512)
    tTu = np.concatenate([-tmat, tmat], axis=0)      # [128, 512]
    tTv = np.concatenate([-smat, smat], axis=0)

    w_c = np.where(c == 0, 1.0, 2.0)
    s_q = 2.0 / (128.0 * 128.0)
    QYc = s_q * w_c[:, None] * np.cos(2 * np.pi * np.outer(c, Y) / 128)   # [32, 128]
    QYs = -s_q * w_c[:, None] * np.sin(2 * np.pi * np.outer(c, Y) / 128)
    Z32 = np.zeros((32, 128))
    # K=64 zero-padded Y-stage consts (K=32 stationary matmuls fault on HW):
    # top variants contract the first 32 K rows (j even), bottom the last 32;
    # replicated x2 down partitions for base-0 / base-64 slicing
    QYcT = np.tile(np.concatenate([QYc, Z32], axis=0), (2, 1))   # [128, 128]
    QYcB = np.tile(np.concatenate([Z32, QYc], axis=0), (2, 1))
    QYsT = np.tile(np.concatenate([QYs, Z32], axis=0), (2, 1))
    QYsB = np.tile(np.concatenate([Z32, QYs], axis=0), (2, 1))

    PRT = np.cos(2 * np.pi * np.outer(FRs, X) / 128)   # [r=64, X=128]
    PIT = np.sin(2 * np.pi * np.outer(FRs, X) / 128)
    PRT[32, :] = 0.0
    PIT[32, :] = 0.0
    # X-first synthesis const [128=(kxR,kxI), 256=(rg 2, X)]
    PPbig = np.zeros((128, 256))
    PPbig[0:64, 0:128] = PRT
    PPbig[64:128, 0:128] = -PIT
    PPbig[0:64, 128:256] = PIT
    PPbig[64:128, 128:256] = PRT

    # direct fr path: fr_i = Rx @ f_i @ Cy^T (pure 2x Fourier upsampling)
    ExRm = np.cos(2 * np.pi * np.outer(kx, x) / 64)
    ExIm = -np.sin(2 * np.pi * np.outer(kx, x) / 64)
    EyRm = np.cos(2 * np.pi * np.outer(c, y) / 64)
    EyIm = -np.sin(2 * np.pi * np.outer(c, y) / 64)
    QRm = s_q * w_c[None, :] * np.cos(2 * np.pi * np.outer(Y, c) / 128)
    QIm = s_q * w_c[None, :] * np.sin(2 * np.pi * np.outer(Y, c) / 128)
    Rx = PRT.T @ ExRm - PIT.T @ ExIm                 # [128, 64] (PRT.T == PR)
    Cy = QRm @ EyRm - QIm @ EyIm                     # [128, 64]
    RxT = Rx.T                                       # [x=64, X=128]
    CyT = np.concatenate([Cy.T, Cy.T], axis=0)       # [128, 128] doubled rows

    ImI = np.concatenate([np.eye(128), -np.eye(128)], axis=1)  # [128, 256]

    bf = lambda a: np.ascontiguousarray(a, dtype=ml_dtypes.bfloat16)
    f32 = lambda a: np.ascontiguousarray(a, dtype=np.float32)
    return dict(ExF=bf(ExF), EyCT=bf(EyCT), EyST=bf(EyST),
                S_sel=bf(S_sel), tTu=f32(tTu), tTv=f32(tTv),
                QYcT=bf(QYcT), QYcB=bf(QYcB), QYsT=bf(QYsT), QYsB=bf(QYsB),
                PPbig=bf(PPbig), RxT=bf(RxT), CyT=bf(CyT), ImI=bf(ImI))


def _rot90_kernel(k):
    # z[..., i, j] = k[..., (-j) mod n, i]
    y = np.swapaxes(k, -2, -1)
    return np.concatenate([y[..., :1], y[..., :0:-1]], axis=-1)


def _symmetric_kernel(k):
    k1 = k
    k2 = _rot90_kernel(k1)
    k3 = _rot90_kernel(k2)
    k4 = _rot90_kernel(k3)
    k5 = np.swapaxes(k1, -2, -1)
    k6 = _rot90_kernel(k5)
    k7 = _rot90_kernel(k6)
    k8 = _rot90_kernel(k7)
    return (k1 + k2 + k3 + k4 + k5 + k6 + k7 + k8) / 8.0


def _prep_k_all(kernel_np):
    """kernel [1,8,16,64,64] -> k_all [128, 2048] conv-layout packed (bf16)."""
    ksym = _symmetric_kernel(kernel_np.astype(np.float64))[0]   # [8,16,64,64]
    K = np.fft.rfft2(ksym).real                                  # [8,16,64,33]
    Kc = np.transpose(K[:, :, :, :32], (0, 1, 3, 2)).copy()      # [i,j,c,kx]
    Kc[:, :, :, 32] = 0.0                                        # kx nyquist
    k_all = np.zeros((128, 2048), dtype=np.float32)
    for i in range(8):
        h, im = i // 4, i % 4
        for j in range(16):
            k_all[im * 32:(im + 1) * 32, j * 128 + h * 64: j * 128 + h * 64 + 64] = Kc[i, j]
    return np.ascontiguousarray(k_all, dtype=ml_dtypes.bfloat16)


# ---------------------------------------------------------------------------
# device program
# ---------------------------------------------------------------------------

def _bcast(ap, n, axis_pos=1):
    """Insert a zero-step broadcast dim of size n into an AP (after partition dim)."""
    dims = list(ap.ap)
    dims.insert(axis_pos, [0, n])
    return AP(ap.tensor, ap.offset, dims)


def _view(ap, offset_elems, dims):
    """Raw AP view on the same tensor: explicit offset (elems) + [step, count] dims."""
    return AP(ap.tensor, ap.offset + offset_elems, dims)


def build_program(reps=1, ablate=(), gps_subs=False, gps_prod8=0, gps_conv=False,
                  sub_pool8=4, pe_sub8=0, **_unused):
    """ablate: subset of {'cross','synth','conv','dma'} to skip (profiling)."""
    nc = bacc.Bacc("TRN2", target_bir_lowering=False)
    consts = _host_consts()

    f_in = nc.dram_tensor("f_in", [B_PER_CORE, C1, 64, 64], F32, kind="ExternalInput")
    k_in = nc.dram_tensor("k_all", [128, 2048], BF16, kind="ExternalInput")
    # transposed output layout [b, X, ch, Y]; host returns .transpose(0,2,1,3) view
    out_sh = nc.dram_tensor("out_sh", [B_PER_CORE, 128, NCH_OUT, 128], F32,
                            kind="ExternalOutput")

    cdr = {name: nc.inline_tensor(arr, name=f"c_{name}") for name, arr in consts.items()}

    with tile.TileContext(nc) as tc:
        with (
            tc.tile_pool(name="cp", bufs=1) as cp,
            tc.tile_pool(name="fld", bufs=1) as fld,     # u_all/v_all/fr_all
            tc.tile_pool(name="wk", bufs=2) as wk,       # small working tiles
            tc.tile_pool(name="mw", bufs=2) as mwp,      # conv wide tiles
            tc.tile_pool(name="wp", bufs=3) as wp,       # cross product blocks
            tc.tile_pool(name="crp", bufs=3) as crp,     # cross output staging
            tc.tile_pool(name="pp", bufs=1, space="PSUM") as pp,
        ):
            # ---- load constants (stage1 deps first, spread over rings) ----
            cs = {}
            const_rings = [nc.scalar, nc.gpsimd]
            order = ['ExF', 'RxT', 'EyCT', 'EyST', 'S_sel', 'CyT',
                     'tTu', 'tTv', 'PPbig', 'QYcT', 'QYcB', 'QYsT', 'QYsB',
                     'ImI']
            for ci, name in enumerate(order):
                if name == 'ImI' and pe_sub8 == 0:
                    continue
                arr = consts[name]
                dt = BF16 if arr.dtype == ml_dtypes.bfloat16 else F32
                t = cp.tile(list(arr.shape), dt, tag=f"c_{name}", name=f"cs_{name}")
                const_rings[ci % 2].dma_start(out=t[:], in_=cdr[name][:])
                cs[name] = t
            k_sb = cp.tile([128, 2048], BF16, tag="k_sb")
            nc.gpsimd.dma_start(out=k_sb[:], in_=k_in[:])

            u_all = fld.tile([128, 16 * 256], BF16, tag="u_all")
            v_all = fld.tile([128, 16 * 256], BF16, tag="v_all")
            fr_all = fld.tile([128, 8 * 256], F32, tag="fr_all")

            dma_tick = [0]
            # weighted ring pattern: SP is otherwise idle, favor it
            ring_pats = {
                0: [nc.sync, nc.scalar, nc.sync, nc.gpsimd, nc.sync, nc.scalar],
                1: [nc.sync, nc.scalar, nc.gpsimd],
                2: [nc.sync, nc.scalar, nc.sync, nc.gpsimd],
                3: [nc.sync, nc.sync, nc.scalar, nc.sync, nc.sync, nc.gpsimd],
            }
            import os as _os
            out_rings = ring_pats[int(_os.environ.get("KRING", "0"))]

            def out_dma(out_ap, in_ap):
                eng = out_rings[dma_tick[0] % len(out_rings)]
                dma_tick[0] += 1
                eng.dma_start(out=out_ap, in_=in_ap)

            prod_tick = [0]

            def prod_eng():
                i = prod_tick[0] % 8
                prod_tick[0] += 1
                return nc.gpsimd if i < gps_prod8 else nc.vector

            sub_tick = [0]

            def sub_eng():
                i = sub_tick[0] % 8
                sub_tick[0] += 1
                return nc.gpsimd if i < sub_pool8 else nc.vector

            def emit_cross_block(gI, gJ, b):
                """cross products for channel groups gI x gJ, one sample.

                One fused [128, 2048] product op per W-block (16 pairs),
                one fused subtract + one 16-channel DMA per off-diag block."""
                W1 = wp.tile([128, 2048], BF16, tag="W1", name="W1")
                # W1[(a, bl, f)] = u[4gI+a] * v[4gJ+bl]
                in0 = _view(u_all[:], gI * 1024 + b * 128,
                            [u_all[:].ap[0], [256, 4], [0, 4], [1, 128]])
                in1 = _view(v_all[:], gJ * 1024 + b * 128,
                            [v_all[:].ap[0], [0, 4], [256, 4], [1, 128]])
                prod_eng().tensor_mul(
                    W1[:].rearrange("p (a c f) -> p a c f", a=4, c=4), in0, in1)
                if gI != gJ:
                    W2 = wp.tile([128, 2048], BF16, tag="W2", name="W2")
                    # W2[(bl, a, f)] = u[4gJ+bl] * v[4gI+a]
                    in0 = _view(u_all[:], gJ * 1024 + b * 128,
                                [u_all[:].ap[0], [256, 4], [0, 4], [1, 128]])
                    in1 = _view(v_all[:], gI * 1024 + b * 128,
                                [v_all[:].ap[0], [0, 4], [256, 4], [1, 128]])
                    prod_eng().tensor_mul(
                        W2[:].rearrange("p (c a f) -> p c a f", c=4, a=4), in0, in1)
                    # cr[(a, bl, f)] = W1[(a, bl, f)] - W2[(bl, a, f)]
                    cr = crp.tile([128, 2048], F32, tag="crb", name="crb")
                    st_i = sub_tick[0] % 8
                    sub_tick[0] += 1
                    if st_i < pe_sub8:
                        # subtract on the TensorEngine: psC = I@W1q - I@W2q',
                        # f32 chunk copies land on ACT/Pool
                        for q in range(4):
                            psC = pp.tile([128, 512], F32, tag="bankC", bufs=2,
                                          name="psC")
                            w2q = _view(W2[:], q * 128,
                                        [W2[:].ap[0], [512, 4], [1, 128]])
                            nc.tensor.matmul(psC[:], cs["ImI"][:, 0:128],
                                             W1[:, q * 512:(q + 1) * 512],
                                             start=True, stop=False)
                            nc.tensor.matmul(
                                psC[:].rearrange("p (c f) -> p c f", c=4),
                                cs["ImI"][:, 128:256], w2q,
                                start=False, stop=True)
                            if q % 2 == 0:
                                nc.scalar.copy(out=cr[:, q * 512:(q + 1) * 512],
                                               in_=psC[:])
                            else:
                                nc.vector.tensor_copy(cr[:, q * 512:(q + 1) * 512],
                                                      psC[:])
                    else:
                        in1s = _view(W2[:], 0,
                                     [W2[:].ap[0], [128, 4], [512, 4], [1, 128]])
                        eng = nc.gpsimd if st_i < pe_sub8 + sub_pool8 else nc.vector
                        eng.tensor_sub(
                            cr[:].rearrange("p (a c f) -> p a c f", a=4, c=4),
                            W1[:].rearrange("p (a c f) -> p a c f", a=4, c=4), in1s)
                    # pair channels are contiguous per a only (stride 15-a
                    # between a rows) -> one 4-channel DMA per a
                    if 'dma' not in ablate:
                        for ai in range(4):
                            pch = 8 + _PAIR_IDX[(4 * gI + ai, 4 * gJ)]
                            out_dma(out_sh[b, :, pch:pch + 4, :],
                                    cr[:, ai * 512:(ai + 1) * 512].rearrange(
                                        "x (c y) -> x c y", c=4))
                else:
                    for ai in range(3):
                        a = 4 * gI + ai
                        cnt = 3 - ai
                        cr = crp.tile([128, 512], F32, tag="cr", name="cr")
                        in0 = _view(W1[:], ai * 512 + (ai + 1) * 128,
                                    [W1[:].ap[0], [128, cnt], [1, 128]])
                        in1 = _view(W1[:], (ai + 1) * 512 + ai * 128,
                                    [W1[:].ap[0], [512, cnt], [1, 128]])
                        sub_eng().tensor_sub(
                            cr[:, 0:cnt * 128].rearrange(
                                "p (cb f) -> p cb f", cb=cnt), in0, in1)
                        pch = 8 + _PAIR_IDX[(a, a + 1)]
                        if 'dma' not in ablate:
                            out_dma(out_sh[b, :, pch:pch + cnt, :],
                                    cr[:, 0:cnt * 128].rearrange("x (c y) -> x c y", c=cnt))

            def emit_stage1(b, st):
                A_ch = []
                T1s = []
                for ip in range(4):
                    fsb = wk.tile([64, 128], F32, tag="fsb", name="fsb")
                    nc.sync.dma_start(
                        out=fsb[:].rearrange("x (i y) -> x i y", i=2),
                        in_=f_in[b, 2 * ip:2 * ip + 2].rearrange("i x y -> x i y"))
                    fsb_bt = wk.tile([64, 128], BF16, tag="fsbb", name="fsbb")
                    nc.vector.tensor_copy(fsb_bt[:], fsb[:])
                    fsb_b = fsb_bt[:]
                    psA = pp.tile([128, 192], F32, tag="bankA", bufs=2, name="psA")
                    nc.tensor.matmul(psA[:], fsb_b, cs["ExF"][:], start=True, stop=True)
                    for iloc in range(2):
                        a_t = wk.tile([64, 192], BF16, tag=f"ach{2*ip+iloc}",
                                      name=f"ach{2*ip+iloc}")
                        nc.vector.tensor_copy(a_t[:], psA[iloc * 64:(iloc + 1) * 64, :])
                        A_ch.append(a_t)
                    # fr path: T1 = [f_i^T Rx^T | f_{i+1}^T Rx^T]  ([y, X] per channel)
                    psT1 = pp.tile([128, 128], F32, tag="bankA", bufs=2, name="psT1")
                    nc.tensor.matmul(psT1[:], fsb_b, cs["RxT"][:], start=True, stop=True)
                    t1sb = wk.tile([128, 128], BF16, tag=f"t1sb{ip}", name=f"t1sb{ip}")
                    nc.scalar.copy(out=t1sb[:], in_=psT1[:])
                    T1s.append(t1sb)
                st['A_ch'] = A_ch
                st['T1s'] = T1s

            def emit_stage2(b, st):
                A_ch = st['A_ch']
                # out free = [F_R(kx64) | F_I(kx64)] per tile
                psFcv = [pp.tile([128, 128], F32, tag=f"bankF{2+h}", name=f"psFcv{h}")
                         for h in range(2)]
                EyC, EyS = cs["EyCT"], cs["EyST"]
                for i in range(8):
                    A_RI = A_ch[i][:, 0:128]     # [A_R | A_I]
                    A_IS = A_ch[i][:, 64:192]    # [A_I | -A_R]
                    h, im = i // 4, i % 4
                    sl = slice(im * 32, (im + 1) * 32)
                    tp = (0, im * 32)
                    nc.tensor.matmul(psFcv[h][sl, :], EyC[0:64, :], A_RI,
                                     start=True, stop=False, tile_position=tp)
                    nc.tensor.matmul(psFcv[h][sl, :], EyS[0:64, :], A_IS,
                                     start=False, stop=True, tile_position=tp)

                Fcv = wk.tile([128, 256], BF16, tag="Fcv", name="Fcv")
                for h in range(2):
                    nc.scalar.copy(out=Fcv[:, h * 64:(h + 1) * 64], in_=psFcv[h][:, 0:64])
                    nc.scalar.copy(out=Fcv[:, 128 + h * 64:128 + (h + 1) * 64],
                                   in_=psFcv[h][:, 64:128])
                st['Fcv'] = Fcv

            def emit_conv(b, st):
                Fcv = st['Fcv']
                Mw = []
                for RI in range(2):
                    m_t = mwp.tile([128, 2048], BF16, tag=f"mw{RI}", name=f"mw{RI}")
                    in0 = _bcast(Fcv[:, RI * 128:(RI + 1) * 128], 16)
                    conv_eng = nc.gpsimd if gps_conv else nc.vector
                    conv_eng.tensor_mul(
                        m_t[:].rearrange("p (j f) -> p j f", j=16),
                        in0,
                        k_sb[:].rearrange("p (j f) -> p j f", j=16))
                    Mw.append(m_t)

                # selector-transpose: psT2 rows 0-63 = acv_I^T [kx, (j,c)],
                # rows 64-127 = acv_R^T (i-sum via stationary=Mw, moving=S_sel)
                psT2 = pp.tile([128, 512], F32, tag="bankT", bufs=1, name="psT2")
                for RI in range(2):
                    rows = slice(64, 128) if RI == 0 else slice(0, 64)
                    for j in range(16):
                        for h in range(2):
                            lhsT = _view(Mw[RI][:], j * 128 + h * 64,
                                         [Mw[RI][:].ap[0], [1, 64]])
                            nc.tensor.matmul(
                                psT2[rows, j * 32:(j + 1) * 32],
                                lhsT, cs["S_sel"][:],
                                start=(h == 0), stop=(h == 1))
                # uncurl in transposed layout: BT [128=(kxR,kxI), 512=(j,c)]
                BTu = wk.tile([128, 512], BF16, tag="BTu", name="BTu")
                BTv = wk.tile([128, 512], BF16, tag="BTv", name="BTv")
                nc.vector.tensor_mul(BTu[:], psT2[:], cs["tTu"][:])
                nc.vector.tensor_mul(BTv[:], psT2[:], cs["tTv"][:])
                st['BT'] = (BTu, BTv)

            def emit_fr(b, st):
                # fr direct: fr_i = (T1_i)^T @ Cy^T via one matmul per channel
                for i in range(8):
                    ip, iloc = i // 2, i % 2
                    t1 = st['T1s'][ip][iloc * 64:(iloc + 1) * 64, :]
                    psUf = pp.tile([128, 128], F32, tag="bankU", bufs=1, name="psUf")
                    nc.tensor.matmul(psUf[:], t1,
                                     cs["CyT"][iloc * 64:(iloc + 1) * 64, :],
                                     start=True, stop=True)
                    nc.scalar.copy(out=fr_all[:, i * 256 + b * 128:i * 256 + (b + 1) * 128],
                                   in_=psUf[:])
                if 'dma' not in ablate:
                    frv = _view(fr_all[:], b * 128,
                                [fr_all[:].ap[0], [256, 8], [1, 128]])
                    out_dma(out_sh[b, :, 0:8, :], frv)

            uvcp_tick = [0]

            def emit_synth_group(b, st, g):
                """synthesize u and v channels 4g..4g+3 via X-first 2-stage DFT."""
                BTu, BTv = st['BT']
                for fi, (BT, dest) in enumerate(((BTu, u_all), (BTv, v_all))):
                    # X-stage: psHT [(4j, c), (rg, X)] = BT-slice^T @ PPbig
                    psHT = pp.tile([128, 256], F32, tag=f"bankF{fi}", name="psHT")
                    nc.tensor.matmul(psHT[:], BT[:, g * 128:(g + 1) * 128],
                                     cs["PPbig"][:], start=True, stop=True)
                    # PE stationary base partition must be 0/32/64: split rows
                    HTa = wk.tile([64, 256], BF16, tag=f"HTa{fi}", name=f"HTa{fi}")
                    HTb = wk.tile([64, 256], BF16, tag=f"HTb{fi}", name=f"HTb{fi}")
                    nc.scalar.copy(out=HTa[:], in_=psHT[0:64, :])
                    nc.scalar.copy(out=HTb[:], in_=psHT[64:128, :])
                    # Y-stage: psF[X, (4j, Y)] = sum_c H_R QYc + H_I QYs
                    # (K=64 zero-padded: K=32 stationary matmuls fault on HW)
                    psF = pp.tile([128, 512], F32, tag=f"bankF{2 + fi}", name="psF")
                    for jl in range(4):
                        HT = HTa if jl < 2 else HTb
                        qc = cs["QYcT"] if jl % 2 == 0 else cs["QYcB"]
                        qs = cs["QYsT"] if jl % 2 == 0 else cs["QYsB"]
                        osl = psF[:, jl * 128:(jl + 1) * 128]
                        nc.tensor.matmul(osl, HT[:, 0:128], qc[0:64, :],
                                         start=True, stop=False)
                        nc.tensor.matmul(osl, HT[:, 128:256], qs[0:64, :],
                                         start=False, stop=True)
                    dsl = _view(dest[:], (4 * g) * 256 + b * 128,
                                [dest[:].ap[0], [256, 4], [1, 128]])
                    nc.scalar.copy(out=dsl, in_=psF[:].rearrange(
                        "p (c y) -> p c y", c=4))
                    uvcp_tick[0] += 1

            for rep in range(reps):
                st = {b: {} for b in range(B_PER_CORE)}
                for b in range(B_PER_CORE):
                    emit_stage1(b, st[b])
                    emit_stage2(b, st[b])
                    if 'conv' in ablate:
                        continue
                    emit_conv(b, st[b])
                if 'conv' in ablate:
                    continue
                if 'synth' in ablate:
                    continue
                for b in range(B_PER_CORE):
                    emit_fr(b, st[b])
                for g in range(4):
                    for b in range(B_PER_CORE):
                        emit_synth_group(b, st[b], g)
                        if 'cross' in ablate:
                            continue
                        for gI in range(g + 1):
                            emit_cross_block(gI, g, b)
    nc.compile()
    return nc


# ---------------------------------------------------------------------------
# entry point
# ---------------------------------------------------------------------------

_PROGRAM = {}


def _get_program(reps=1, ablate=(), **kw):
    global _PROGRAM
    import os
    if 'gps_subs' not in kw:
        kw['gps_subs'] = os.environ.get("KGPS", "0") == "1"
    if 'gps_prod8' not in kw:
        kw['gps_prod8'] = int(os.environ.get("KGPSP", "0"))
    if 'gps_conv' not in kw:
        kw['gps_conv'] = os.environ.get("KGPSC", "0") == "1"
    if 'sub_pool8' not in kw:
        kw['sub_pool8'] = int(os.environ.get("KSUBP", "8"))
    if 'pe_sub8' not in kw:
        kw['pe_sub8'] = int(os.environ.get("KPESUB", "0"))
    key = (reps, tuple(sorted(ablate)), tuple(sorted(kw.items())))
    if key not in _PROGRAM:
        _PROGRAM[key] = build_program(reps, ablate=ablate, **kw)
    return _PROGRAM[key]


LAST_EXEC_NS = None
LAST_RESULT = None


def kernel(f, kernel):
    global LAST_EXEC_NS, LAST_RESULT
    f = np.ascontiguousarray(f, dtype=np.float32)
    k_all = _prep_k_all(np.asarray(kernel))
    nc = _get_program()
    in_maps = [
        {"f_in": f[2 * c:2 * c + 2], "k_all": k_all} for c in range(N_CORES)
    ]
    import os
    trace = bool(os.environ.get("KERNEL_TRACE"))
    res = run_bass_kernel_spmd(nc, in_maps, list(range(N_CORES)), trace=trace)
    LAST_RESULT = res
    if res.exec_time_ns is not None:
        LAST_EXEC_NS = res.exec_time_ns
    out = np.concatenate([res.results[c]["out_sh"] for c in range(N_CORES)], axis=0)
    # device layout is [b, X, ch, Y]; return the [b, ch, X, Y] view
    return out.transpose(0, 2, 1, 3)


# revision 89
# speedup vs baseline: 1.0050x; 1.0050x over previous
"""Trainium2 Bass kernel for nn_EquivariantLayer (spectral equivariant layer).

Strategy (data-parallel over batch, 2 samples/core x 8 cores):
  All FFTs are expressed as real DFT matmuls on the TensorEngine with layouts
  chosen so no corner-turn transposes are ever needed:

    stage1:  A = f^T @ [ExR^T | ExI^T]          (contract x; out [y, (RI,kx)])
    stage2:  F = Ey @ A                          (contract y; out [c, kx], c-major)
    conv:    M = F (*) K elementwise (K = rfft2(sym kernel) is REAL since the
             symmetrized kernel is D4-symmetric); the i-reduction runs as a
             "selector-transpose" matmul (stationary = M-slice, moving =
             selector) so the conv spectrum lands TRANSPOSED [kx, (j, c)]
    uncurl:  TO_U = i*t, TO_V = i*s are pure-imaginary -> one fused real mult
             per field over the partition-stacked [acv_I; acv_R] PSUM tile
    synth:   X-first 2-stage iDFT: psHT[(4j,c),(RI,X)] = BT^T @ PPbig, then
             psF[X,(4j,Y)] = sum_c H_R QYc + H_I QYs with K=64 zero-padded
             stationaries (K=32 stationary matmuls fault on TRN2 HW)
    cross:   u_a v_b - u_b v_a: fused [128,2048] bf16 products (16 pairs per
             DVE op via zero-step broadcast APs), f32 subtract on gpsimd/DVE

  The compute pipeline runs in bf16 (PE at 1 cycle/row, DVE 2x mode); PSUM
  accumulation stays fp32 and the final subtraction materializes fp32 output.
  Output DMAs round-robin over the SP / ACT / SWDGE rings (SP-weighted).
  HW constraints honored: gpsimd never touches PSUM; tensor_tensor operands
  share a partition base; PE stationary bases are 0/32/64 with K >= 64.

Output [16, 128, 128, 128] f32 (~134 MB) dominates traffic (memory regime).
"""
import sys
import numpy as np
import ml_dtypes

if '/opt/trn_rl_repo' not in sys.path:
    sys.path.insert(0, '/opt/trn_rl_repo')

import concourse.bass as bass
from concourse import bacc
import concourse.mybir as mybir
import concourse.tile as tile
from concourse.bass import AP
from concourse.bass_utils import run_bass_kernel_spmd

F32 = mybir.dt.float32
BF16 = mybir.dt.bfloat16
N_CORES = 8
B_PER_CORE = 2
C1, C2, N1, N2 = 8, 16, 64, 128
NCH_OUT = 128  # 8 fr + 120 cross

I_IDX, J_IDX = np.triu_indices(C2, 1)
_PAIR_IDX = {}
for _p, (_a, _b) in enumerate(zip(I_IDX, J_IDX)):
    _PAIR_IDX[(int(_a), int(_b))] = _p


# ---------------------------------------------------------------------------
# host-side constant construction
# ---------------------------------------------------------------------------

def _host_consts():
    x = np.arange(64)
    kx = np.arange(64)
    c = np.arange(32)
    y = np.arange(64)
    X = np.arange(128)
    Y = np.arange(128)

    FRs = np.where(kx <= 32, kx, kx - 64).astype(np.float64)  # signed row freq

    ExR = np.cos(2 * np.pi * np.outer(kx, x) / 64)   # [kx, x]
    ExI = -np.sin(2 * np.pi * np.outer(kx, x) / 64)
    # [A_R | A_I | -A_R] so stage2 fuses R/I into two matmuls
    ExF = np.concatenate([ExR.T, ExI.T, -ExR.T], axis=1)   # [x, 192]

    # F_R = C A_R + S A_I ; F_I = C A_I + S (-A_R)   (C=cos, S=sin)
    # replicated x2 down partitions so base-0 and base-64 slices both exist
    EyCT = np.tile(np.cos(2 * np.pi * np.outer(c, y) / 64).T, (2, 1))  # [128, 32]
    EyST = np.tile(np.sin(2 * np.pi * np.outer(c, y) / 64).T, (2, 1))

    S_sel = np.zeros((128, 32))
    for im in range(4):
        S_sel[im * 32 + np.arange(32), np.arange(32)] = 1.0

    den = FRs[None, :] ** 2 + c[:, None].astype(np.float64) ** 2
    den[0, 0] = 1.0
    t_u = c[:, None] / den                           # [32, 64]
    s_v = -FRs[None, :] / den

    # uncurl consts in transposed [kx, (j-rep 16, c 32)] layout, partition-
    # aligned with psT2 = [acv_I (rows 0-63); acv_R (rows 64-127)]:
    #   BTu = psT2 * [-t; +t],  BTv = psT2 * [-s; +s]
    tmat = np.tile(t_u.T[:, None, :], (1, 16, 1)).reshape(64, 512)
    smat = np.tile(s_v.T[:, None, :], (1, 16, 1)).reshape(64, 512)
    tTu = np.concatenate([-tmat, tmat], axis=0)      # [128, 512]
    tTv = np.concatenate([-smat, smat], axis=0)

    w_c = np.where(c == 0, 1.0, 2.0)
    s_q = 2.0 / (128.0 * 128.0)
    QYc = s_q * w_c[:, None] * np.cos(2 * np.pi * np.outer(c, Y) / 128)   # [32, 128]
    QYs = -s_q * w_c[:, None] * np.sin(2 * np.pi * np.outer(c, Y) / 128)
    Z32 = np.zeros((32, 128))
    # K=64 zero-padded Y-stage consts (K=32 stationary matmuls fault on HW):
    # top variants contract the first 32 K rows (j even), bottom the last 32;
    # replicated x2 down partitions for base-0 / base-64 slicing
    QYcT = np.tile(np.concatenate([QYc, Z32], axis=0), (2, 1))   # [128, 128]
    QYcB = np.tile(np.concatenate([Z32, QYc], axis=0), (2, 1))
    QYsT = np.tile(np.concatenate([QYs, Z32], axis=0), (2, 1))
    QYsB = np.tile(np.concatenate([Z32, QYs], axis=0), (2, 1))

    PRT = np.cos(2 * np.pi * np.outer(FRs, X) / 128)   # [r=64, X=128]
    PIT = np.sin(2 * np.pi * np.outer(FRs, X) / 128)
    PRT[32, :] = 0.0
    PIT[32, :] = 0.0
    # X-first synthesis const [128=(kxR,kxI), 256=(rg 2, X)]
    PPbig = np.zeros((128, 256))
    PPbig[0:64, 0:128] = PRT
    PPbig[64:128, 0:128] = -PIT
    PPbig[0:64, 128:256] = PIT
    PPbig[64:128, 128:256] = PRT

    # direct fr path: fr_i = Rx @ f_i @ Cy^T (pure 2x Fourier upsampling)
    ExRm = np.cos(2 * np.pi * np.outer(kx, x) / 64)
    ExIm = -np.sin(2 * np.pi * np.outer(kx, x) / 64)
    EyRm = np.cos(2 * np.pi * np.outer(c, y) / 64)
    EyIm = -np.sin(2 * np.pi * np.outer(c, y) / 64)
    QRm = s_q * w_c[None, :] * np.cos(2 * np.pi * np.outer(Y, c) / 128)
    QIm = s_q * w_c[None, :] * np.sin(2 * np.pi * np.outer(Y, c) / 128)
    Rx = PRT.T @ ExRm - PIT.T @ ExIm                 # [128, 64] (PRT.T == PR)
    Cy = QRm @ EyRm - QIm @ EyIm                     # [128, 64]
    RxT = Rx.T                                       # [x=64, X=128]
    CyT = np.concatenate([Cy.T, Cy.T], axis=0)       # [128, 128] doubled rows

    ImI = np.concatenate([np.eye(128), -np.eye(128)], axis=1)  # [128, 256]

    bf = lambda a: np.ascontiguousarray(a, dtype=ml_dtypes.bfloat16)
    f32 = lambda a: np.ascontiguousarray(a, dtype=np.float32)
    return dict(ExF=bf(ExF), EyCT=bf(EyCT), EyST=bf(EyST),
                S_sel=bf(S_sel), tTu=f32(tTu), tTv=f32(tTv),
                QYcT=bf(QYcT), QYcB=bf(QYcB), QYsT=bf(QYsT), QYsB=bf(QYsB),
                PPbig=bf(PPbig), RxT=bf(RxT), CyT=bf(CyT), ImI=bf(ImI))


def _rot90_kernel(k):
    # z[..., i, j] = k[..., (-j) mod n, i]
    y = np.swapaxes(k, -2, -1)
    return np.concatenate([y[..., :1], y[..., :0:-1]], axis=-1)


def _symmetric_kernel(k):
    k1 = k
    k2 = _rot90_kernel(k1)
    k3 = _rot90_kernel(k2)
    k4 = _rot90_kernel(k3)
    k5 = np.swapaxes(k1, -2, -1)
    k6 = _rot90_kernel(k5)
    k7 = _rot90_kernel(k6)
    k8 = _rot90_kernel(k7)
    return (k1 + k2 + k3 + k4 + k5 + k6 + k7 + k8) / 8.0


def _prep_k_all(kernel_np):
    """kernel [1,8,16,64,64] -> k_all [128, 2048] conv-layout packed (bf16)."""
    ksym = _symmetric_kernel(kernel_np.astype(np.float64))[0]   # [8,16,64,64]
    K = np.fft.rfft2(ksym).real                                  # [8,16,64,33]
    Kc = np.transpose(K[:, :, :, :32], (0, 1, 3, 2)).copy()      # [i,j,c,kx]
    Kc[:, :, :, 32] = 0.0                                        # kx nyquist
    k_all = np.zeros((128, 2048), dtype=np.float32)
    for i in range(8):
        h, im = i // 4, i % 4
        for j in range(16):
            k_all[im * 32:(im + 1) * 32, j * 128 + h * 64: j * 128 + h * 64 + 64] = Kc[i, j]
    return np.ascontiguousarray(k_all, dtype=ml_dtypes.bfloat16)


# ---------------------------------------------------------------------------
# device program
# ---------------------------------------------------------------------------

def _bcast(ap, n, axis_pos=1):
    """Insert a zero-step broadcast dim of size n into an AP (after partition dim)."""
    dims = list(ap.ap)
    dims.insert(axis_pos, [0, n])
    return AP(ap.tensor, ap.offset, dims)


def _view(ap, offset_elems, dims):
    """Raw AP view on the same tensor: explicit offset (elems) + [step, count] dims."""
    return AP(ap.tensor, ap.offset + offset_elems, dims)


def build_program(reps=1, ablate=(), gps_subs=False, gps_prod8=0, gps_conv=False,
                  sub_pool8=4, pe_sub8=0, **_unused):
    """ablate: subset of {'cross','synth','conv','dma'} to skip (profiling)."""
    nc = bacc.Bacc("TRN2", target_bir_lowering=False)
    consts = _host_consts()

    f_in = nc.dram_tensor("f_in", [B_PER_CORE, C1, 64, 64], F32, kind="ExternalInput")
    k_in = nc.dram_tensor("k_all", [128, 2048], BF16, kind="ExternalInput")
    # transposed output layout [b, X, ch, Y]; host returns .transpose(0,2,1,3) view
    out_sh = nc.dram_tensor("out_sh", [B_PER_CORE, 128, NCH_OUT, 128], F32,
                            kind="ExternalOutput")

    cdr = {name: nc.inline_tensor(arr, name=f"c_{name}") for name, arr in consts.items()}

    with tile.TileContext(nc) as tc:
        with (
            tc.tile_pool(name="cp", bufs=1) as cp,
            tc.tile_pool(name="fld", bufs=1) as fld,     # u_all/v_all/fr_all
            tc.tile_pool(name="wk", bufs=3) as wk,       # small working tiles
            tc.tile_pool(name="mw", bufs=2) as mwp,      # conv wide tiles
            tc.tile_pool(name="wp", bufs=3) as wp,       # cross product blocks
            tc.tile_pool(name="crp", bufs=3) as crp,     # cross output staging
            tc.tile_pool(name="pp", bufs=1, space="PSUM") as pp,
        ):
            # ---- load constants (stage1 deps first, spread over rings) ----
            cs = {}
            const_rings = [nc.scalar, nc.gpsimd]
            order = ['ExF', 'RxT', 'EyCT', 'EyST', 'S_sel', 'CyT',
                     'tTu', 'tTv', 'PPbig', 'QYcT', 'QYcB', 'QYsT', 'QYsB',
                     'ImI']
            for ci, name in enumerate(order):
                if name == 'ImI' and pe_sub8 == 0:
                    continue
                arr = consts[name]
                dt = BF16 if arr.dtype == ml_dtypes.bfloat16 else F32
                t = cp.tile(list(arr.shape), dt, tag=f"c_{name}", name=f"cs_{name}")
                const_rings[ci % 2].dma_start(out=t[:], in_=cdr[name][:])
                cs[name] = t
            k_sb = cp.tile([128, 2048], BF16, tag="k_sb")
            nc.gpsimd.dma_start(out=k_sb[:], in_=k_in[:])

            u_all = fld.tile([128, 16 * 256], BF16, tag="u_all")
            v_all = fld.tile([128, 16 * 256], BF16, tag="v_all")
            fr_all = fld.tile([128, 8 * 256], F32, tag="fr_all")

            dma_tick = [0]
            # weighted ring pattern: SP is otherwise idle, favor it
            ring_pats = {
                0: [nc.sync, nc.scalar, nc.sync, nc.gpsimd, nc.sync, nc.scalar],
                1: [nc.sync, nc.scalar, nc.gpsimd],
                2: [nc.sync, nc.scalar, nc.sync, nc.gpsimd],
                3: [nc.sync, nc.sync, nc.scalar, nc.sync, nc.sync, nc.gpsimd],
                4: [nc.sync, nc.gpsimd, nc.sync, nc.gpsimd, nc.sync, nc.scalar],
            }
            import os as _os
            out_rings = ring_pats[int(_os.environ.get("KRING", "0"))]

            def out_dma(out_ap, in_ap):
                eng = out_rings[dma_tick[0] % len(out_rings)]
                dma_tick[0] += 1
                eng.dma_start(out=out_ap, in_=in_ap)

            prod_tick = [0]

            def prod_eng():
                i = prod_tick[0] % 8
                prod_tick[0] += 1
                return nc.gpsimd if i < gps_prod8 else nc.vector

            sub_tick = [0]

            def sub_eng():
                i = sub_tick[0] % 8
                sub_tick[0] += 1
                return nc.gpsimd if i < sub_pool8 else nc.vector

            def emit_cross_block(gI, gJ, b):
                """cross products for channel groups gI x gJ, one sample.

                One fused [128, 2048] product op per W-block (16 pairs),
                one fused subtract + one 16-channel DMA per off-diag block."""
                W1 = wp.tile([128, 2048], BF16, tag="W1", name="W1")
                # W1[(a, bl, f)] = u[4gI+a] * v[4gJ+bl]
                in0 = _view(u_all[:], gI * 1024 + b * 128,
                            [u_all[:].ap[0], [256, 4], [0, 4], [1, 128]])
                in1 = _view(v_all[:], gJ * 1024 + b * 128,
                            [v_all[:].ap[0], [0, 4], [256, 4], [1, 128]])
                prod_eng().tensor_mul(
                    W1[:].rearrange("p (a c f) -> p a c f", a=4, c=4), in0, in1)
                if gI != gJ:
                    W2 = wp.tile([128, 2048], BF16, tag="W2", name="W2")
                    # W2[(bl, a, f)] = u[4gJ+bl] * v[4gI+a]
                    in0 = _view(u_all[:], gJ * 1024 + b * 128,
                                [u_all[:].ap[0], [256, 4], [0, 4], [1, 128]])
                    in1 = _view(v_all[:], gI * 1024 + b * 128,
                                [v_all[:].ap[0], [0, 4], [256, 4], [1, 128]])
                    prod_eng().tensor_mul(
                        W2[:].rearrange("p (c a f) -> p c a f", c=4, a=4), in0, in1)
                    # cr[(a, bl, f)] = W1[(a, bl, f)] - W2[(bl, a, f)]
                    cr = crp.tile([128, 2048], F32, tag="crb", name="crb")
                    st_i = sub_tick[0] % 8
                    sub_tick[0] += 1
                    if st_i < pe_sub8:
                        # subtract on the TensorEngine: psC = I@W1q - I@W2q',
                        # f32 chunk copies land on ACT/Pool
                        for q in range(4):
                            psC = pp.tile([128, 512], F32, tag="bankC", bufs=2,
                                          name="psC")
                            w2q = _view(W2[:], q * 128,
                                        [W2[:].ap[0], [512, 4], [1, 128]])
                            nc.tensor.matmul(psC[:], cs["ImI"][:, 0:128],
                                             W1[:, q * 512:(q + 1) * 512],
                                             start=True, stop=False)
                            nc.tensor.matmul(
                                psC[:].rearrange("p (c f) -> p c f", c=4),
                                cs["ImI"][:, 128:256], w2q,
                                start=False, stop=True)
                            if q % 2 == 0:
                                nc.scalar.copy(out=cr[:, q * 512:(q + 1) * 512],
                                               in_=psC[:])
                            else:
                                nc.vector.tensor_copy(cr[:, q * 512:(q + 1) * 512],
                                                      psC[:])
                    else:
                        in1s = _view(W2[:], 0,
                                     [W2[:].ap[0], [128, 4], [512, 4], [1, 128]])
                        eng = nc.gpsimd if st_i < pe_sub8 + sub_pool8 else nc.vector
                        eng.tensor_sub(
                            cr[:].rearrange("p (a c f) -> p a c f", a=4, c=4),
                            W1[:].rearrange("p (a c f) -> p a c f", a=4, c=4), in1s)
                    # pair channels are contiguous per a only (stride 15-a
                    # between a rows) -> one 4-channel DMA per a
                    if 'dma' not in ablate:
                        for ai in range(4):
                            pch = 8 + _PAIR_IDX[(4 * gI + ai, 4 * gJ)]
                            out_dma(out_sh[b, :, pch:pch + 4, :],
                                    cr[:, ai * 512:(ai + 1) * 512].rearrange(
                                        "x (c y) -> x c y", c=4))
                else:
                    for ai in range(3):
                        a = 4 * gI + ai
                        cnt = 3 - ai
                        cr = crp.tile([128, 512], F32, tag="cr", name="cr")
                        in0 = _view(W1[:], ai * 512 + (ai + 1) * 128,
                                    [W1[:].ap[0], [128, cnt], [1, 128]])
                        in1 = _view(W1[:], (ai + 1) * 512 + ai * 128,
                                    [W1[:].ap[0], [512, cnt], [1, 128]])
                        sub_eng().tensor_sub(
                            cr[:, 0:cnt * 128].rearrange(
                                "p (cb f) -> p cb f", cb=cnt), in0, in1)
                        pch = 8 + _PAIR_IDX[(a, a + 1)]
                        if 'dma' not in ablate:
                            out_dma(out_sh[b, :, pch:pch + cnt, :],
                                    cr[:, 0:cnt * 128].rearrange("x (c y) -> x c y", c=cnt))

            def emit_stage1(b, st):
                A_ch = []
                T1s = []
                for ip in range(4):
                    fsb = wk.tile([64, 128], F32, tag="fsb", name="fsb")
                    nc.sync.dma_start(
                        out=fsb[:].rearrange("x (i y) -> x i y", i=2),
                        in_=f_in[b, 2 * ip:2 * ip + 2].rearrange("i x y -> x i y"))
                    fsb_bt = wk.tile([64, 128], BF16, tag="fsbb", name="fsbb")
                    nc.vector.tensor_copy(fsb_bt[:], fsb[:])
                    fsb_b = fsb_bt[:]
                    psA = pp.tile([128, 192], F32, tag="bankA", bufs=2, name="psA")
                    nc.tensor.matmul(psA[:], fsb_b, cs["ExF"][:], start=True, stop=True)
                    for iloc in range(2):
                        a_t = wk.tile([64, 192], BF16, tag=f"ach{2*ip+iloc}",
                                      name=f"ach{2*ip+iloc}")
                        nc.vector.tensor_copy(a_t[:], psA[iloc * 64:(iloc + 1) * 64, :])
                        A_ch.append(a_t)
                    # fr path: T1 = [f_i^T Rx^T | f_{i+1}^T Rx^T]  ([y, X] per channel)
                    psT1 = pp.tile([128, 128], F32, tag="bankA", bufs=2, name="psT1")
                    nc.tensor.matmul(psT1[:], fsb_b, cs["RxT"][:], start=True, stop=True)
                    t1sb = wk.tile([128, 128], BF16, tag=f"t1sb{ip}", name=f"t1sb{ip}")
                    nc.scalar.copy(out=t1sb[:], in_=psT1[:])
                    T1s.append(t1sb)
                st['A_ch'] = A_ch
                st['T1s'] = T1s

            def emit_stage2(b, st):
                A_ch = st['A_ch']
                # out free = [F_R(kx64) | F_I(kx64)] per tile
                psFcv = [pp.tile([128, 128], F32, tag=f"bankF{2+h}", name=f"psFcv{h}")
                         for h in range(2)]
                EyC, EyS = cs["EyCT"], cs["EyST"]
                for i in range(8):
                    A_RI = A_ch[i][:, 0:128]     # [A_R | A_I]
                    A_IS = A_ch[i][:, 64:192]    # [A_I | -A_R]
                    h, im = i // 4, i % 4
                    sl = slice(im * 32, (im + 1) * 32)
                    tp = (0, im * 32)
                    nc.tensor.matmul(psFcv[h][sl, :], EyC[0:64, :], A_RI,
                                     start=True, stop=False, tile_position=tp)
                    nc.tensor.matmul(psFcv[h][sl, :], EyS[0:64, :], A_IS,
                                     start=False, stop=True, tile_position=tp)

                Fcv = wk.tile([128, 256], BF16, tag="Fcv", name="Fcv")
                for h in range(2):
                    nc.scalar.copy(out=Fcv[:, h * 64:(h + 1) * 64], in_=psFcv[h][:, 0:64])
                    nc.scalar.copy(out=Fcv[:, 128 + h * 64:128 + (h + 1) * 64],
                                   in_=psFcv[h][:, 64:128])
                st['Fcv'] = Fcv

            def emit_conv(b, st):
                Fcv = st['Fcv']
                Mw = []
                for RI in range(2):
                    m_t = mwp.tile([128, 2048], BF16, tag=f"mw{RI}", name=f"mw{RI}")
                    in0 = _bcast(Fcv[:, RI * 128:(RI + 1) * 128], 16)
                    conv_eng = nc.gpsimd if gps_conv else nc.vector
                    conv_eng.tensor_mul(
                        m_t[:].rearrange("p (j f) -> p j f", j=16),
                        in0,
                        k_sb[:].rearrange("p (j f) -> p j f", j=16))
                    Mw.append(m_t)

                # selector-transpose: psT2 rows 0-63 = acv_I^T [kx, (j,c)],
                # rows 64-127 = acv_R^T (i-sum via stationary=Mw, moving=S_sel)
                psT2 = pp.tile([128, 512], F32, tag="bankT", bufs=1, name="psT2")
                for RI in range(2):
                    rows = slice(64, 128) if RI == 0 else slice(0, 64)
                    for j in range(16):
                        for h in range(2):
                            lhsT = _view(Mw[RI][:], j * 128 + h * 64,
                                         [Mw[RI][:].ap[0], [1, 64]])
                            nc.tensor.matmul(
                                psT2[rows, j * 32:(j + 1) * 32],
                                lhsT, cs["S_sel"][:],
                                start=(h == 0), stop=(h == 1))
                # uncurl in transposed layout: BT [128=(kxR,kxI), 512=(j,c)]
                BTu = wk.tile([128, 512], BF16, tag="BTu", name="BTu")
                BTv = wk.tile([128, 512], BF16, tag="BTv", name="BTv")
                nc.vector.tensor_mul(BTu[:], psT2[:], cs["tTu"][:])
                nc.vector.tensor_mul(BTv[:], psT2[:], cs["tTv"][:])
                st['BT'] = (BTu, BTv)

            def emit_fr(b, st):
                # fr direct: fr_i = (T1_i)^T @ Cy^T via one matmul per channel
                for i in range(8):
                    ip, iloc = i // 2, i % 2
                    t1 = st['T1s'][ip][iloc * 64:(iloc + 1) * 64, :]
                    psUf = pp.tile([128, 128], F32, tag="bankU", bufs=1, name="psUf")
                    nc.tensor.matmul(psUf[:], t1,
                                     cs["CyT"][iloc * 64:(iloc + 1) * 64, :],
                                     start=True, stop=True)
                    nc.scalar.copy(out=fr_all[:, i * 256 + b * 128:i * 256 + (b + 1) * 128],
                                   in_=psUf[:])
                if 'dma' not in ablate:
                    frv = _view(fr_all[:], b * 128,
                                [fr_all[:].ap[0], [256, 8], [1, 128]])
                    out_dma(out_sh[b, :, 0:8, :], frv)

            uvcp_tick = [0]

            def emit_synth_group(b, st, g):
                """synthesize u and v channels 4g..4g+3 via X-first 2-stage DFT."""
                BTu, BTv = st['BT']
                for fi, (BT, dest) in enumerate(((BTu, u_all), (BTv, v_all))):
                    # X-stage: psHT [(4j, c), (rg, X)] = BT-slice^T @ PPbig
                    psHT = pp.tile([128, 256], F32, tag=f"bankF{fi}", name="psHT")
                    nc.tensor.matmul(psHT[:], BT[:, g * 128:(g + 1) * 128],
                                     cs["PPbig"][:], start=True, stop=True)
                    # PE stationary base partition must be 0/32/64: split rows
                    HTa = wk.tile([64, 256], BF16, tag=f"HTa{fi}", name=f"HTa{fi}")
                    HTb = wk.tile([64, 256], BF16, tag=f"HTb{fi}", name=f"HTb{fi}")
                    nc.scalar.copy(out=HTa[:], in_=psHT[0:64, :])
                    nc.scalar.copy(out=HTb[:], in_=psHT[64:128, :])
                    # Y-stage: psF[X, (4j, Y)] = sum_c H_R QYc + H_I QYs
                    # (K=64 zero-padded: K=32 stationary matmuls fault on HW)
                    psF = pp.tile([128, 512], F32, tag=f"bankF{2 + fi}", name="psF")
                    for jl in range(4):
                        HT = HTa if jl < 2 else HTb
                        qc = cs["QYcT"] if jl % 2 == 0 else cs["QYcB"]
                        qs = cs["QYsT"] if jl % 2 == 0 else cs["QYsB"]
                        osl = psF[:, jl * 128:(jl + 1) * 128]
                        nc.tensor.matmul(osl, HT[:, 0:128], qc[0:64, :],
                                         start=True, stop=False)
                        nc.tensor.matmul(osl, HT[:, 128:256], qs[0:64, :],
                                         start=False, stop=True)
                    dsl = _view(dest[:], (4 * g) * 256 + b * 128,
                                [dest[:].ap[0], [256, 4], [1, 128]])
                    nc.scalar.copy(out=dsl, in_=psF[:].rearrange(
                        "p (c y) -> p c y", c=4))
                    uvcp_tick[0] += 1

            for rep in range(reps):
                st = {b: {} for b in range(B_PER_CORE)}
                for b in range(B_PER_CORE):
                    emit_stage1(b, st[b])
                    emit_stage2(b, st[b])
                    if 'conv' in ablate:
                        continue
                    emit_conv(b, st[b])
                if 'conv' in ablate:
                    continue
                if 'synth' in ablate:
                    continue
                for b in range(B_PER_CORE):
                    emit_fr(b, st[b])
                for g in range(4):
                    for b in range(B_PER_CORE):
                        emit_synth_group(b, st[b], g)
                        if 'cross' in ablate:
                            continue
                        for gI in range(g + 1):
                            emit_cross_block(gI, g, b)
    nc.compile()
    return nc


# ---------------------------------------------------------------------------
# entry point
# ---------------------------------------------------------------------------

_PROGRAM = {}


def _get_program(reps=1, ablate=(), **kw):
    global _PROGRAM
    import os
    if 'gps_subs' not in kw:
        kw['gps_subs'] = os.environ.get("KGPS", "0") == "1"
    if 'gps_prod8' not in kw:
        kw['gps_prod8'] = int(os.environ.get("KGPSP", "0"))
    if 'gps_conv' not in kw:
        kw['gps_conv'] = os.environ.get("KGPSC", "0") == "1"
    if 'sub_pool8' not in kw:
        kw['sub_pool8'] = int(os.environ.get("KSUBP", "8"))
    if 'pe_sub8' not in kw:
        kw['pe_sub8'] = int(os.environ.get("KPESUB", "0"))
    key = (reps, tuple(sorted(ablate)), tuple(sorted(kw.items())))
    if key not in _PROGRAM:
        _PROGRAM[key] = build_program(reps, ablate=ablate, **kw)
    return _PROGRAM[key]


LAST_EXEC_NS = None
LAST_RESULT = None


def kernel(f, kernel):
    global LAST_EXEC_NS, LAST_RESULT
    f = np.ascontiguousarray(f, dtype=np.float32)
    k_all = _prep_k_all(np.asarray(kernel))
    nc = _get_program()
    in_maps = [
        {"f_in": f[2 * c:2 * c + 2], "k_all": k_all} for c in range(N_CORES)
    ]
    import os
    trace = bool(os.environ.get("KERNEL_TRACE"))
    res = run_bass_kernel_spmd(nc, in_maps, list(range(N_CORES)), trace=trace)
    LAST_RESULT = res
    if res.exec_time_ns is not None:
        LAST_EXEC_NS = res.exec_time_ns
    out = np.concatenate([res.results[c]["out_sh"] for c in range(N_CORES)], axis=0)
    # device layout is [b, X, ch, Y]; return the [b, ch, X, Y] view
    return out.transpose(0, 2, 1, 3)


# revision 93
# speedup vs baseline: 1.0232x; 1.0181x over previous
"""Trainium2 Bass kernel for nn_EquivariantLayer (spectral equivariant layer).

Strategy (data-parallel over batch, 2 samples/core x 8 cores):
  All FFTs are expressed as real DFT matmuls on the TensorEngine with layouts
  chosen so no corner-turn transposes are ever needed:

    stage1:  A = f^T @ [ExR^T | ExI^T]          (contract x; out [y, (RI,kx)])
    stage2:  F = Ey @ A                          (contract y; out [c, kx], c-major)
    conv:    M = F (*) K elementwise (K = rfft2(sym kernel) is REAL since the
             symmetrized kernel is D4-symmetric); the i-reduction runs as a
             "selector-transpose" matmul (stationary = M-slice, moving =
             selector) so the conv spectrum lands TRANSPOSED [kx, (j, c)]
    uncurl:  TO_U = i*t, TO_V = i*s are pure-imaginary -> one fused real mult
             per field over the partition-stacked [acv_I; acv_R] PSUM tile
    synth:   X-first 2-stage iDFT: psHT[(4j,c),(RI,X)] = BT^T @ PPbig, then
             psF[X,(4j,Y)] = sum_c H_R QYc + H_I QYs with K=64 zero-padded
             stationaries (K=32 stationary matmuls fault on TRN2 HW)
    cross:   u_a v_b - u_b v_a: fused [128,2048] bf16 products (16 pairs per
             DVE op via zero-step broadcast APs), f32 subtract on gpsimd/DVE

  The compute pipeline runs in bf16 (PE at 1 cycle/row, DVE 2x mode); PSUM
  accumulation stays fp32 and the final subtraction materializes fp32 output.
  Output DMAs round-robin over the SP / ACT / SWDGE rings (SP-weighted).
  HW constraints honored: gpsimd never touches PSUM; tensor_tensor operands
  share a partition base; PE stationary bases are 0/32/64 with K >= 64.

Output [16, 128, 128, 128] f32 (~134 MB) dominates traffic (memory regime).
"""
import sys
import numpy as np
import ml_dtypes

if '/opt/trn_rl_repo' not in sys.path:
    sys.path.insert(0, '/opt/trn_rl_repo')

import concourse.bass as bass
from concourse import bacc
import concourse.mybir as mybir
import concourse.tile as tile
from concourse.bass import AP
from concourse.bass_utils import run_bass_kernel_spmd

F32 = mybir.dt.float32
BF16 = mybir.dt.bfloat16
N_CORES = 8
B_PER_CORE = 2
C1, C2, N1, N2 = 8, 16, 64, 128
NCH_OUT = 128  # 8 fr + 120 cross

I_IDX, J_IDX = np.triu_indices(C2, 1)
_PAIR_IDX = {}
for _p, (_a, _b) in enumerate(zip(I_IDX, J_IDX)):
    _PAIR_IDX[(int(_a), int(_b))] = _p


# ---------------------------------------------------------------------------
# host-side constant construction
# ---------------------------------------------------------------------------

def _host_consts():
    x = np.arange(64)
    kx = np.arange(64)
    c = np.arange(32)
    y = np.arange(64)
    X = np.arange(128)
    Y = np.arange(128)

    FRs = np.where(kx <= 32, kx, kx - 64).astype(np.float64)  # signed row freq

    ExR = np.cos(2 * np.pi * np.outer(kx, x) / 64)   # [kx, x]
    ExI = -np.sin(2 * np.pi * np.outer(kx, x) / 64)
    # [A_R | A_I | -A_R] so stage2 fuses R/I into two matmuls
    ExF = np.concatenate([ExR.T, ExI.T, -ExR.T], axis=1)   # [x, 192]

    # F_R = C A_R + S A_I ; F_I = C A_I + S (-A_R)   (C=cos, S=sin)
    # replicated x2 down partitions so base-0 and base-64 slices both exist
    EyCT = np.tile(np.cos(2 * np.pi * np.outer(c, y) / 64).T, (2, 1))  # [128, 32]
    EyST = np.tile(np.sin(2 * np.pi * np.outer(c, y) / 64).T, (2, 1))

    S_sel = np.zeros((128, 32))
    for im in range(4):
        S_sel[im * 32 + np.arange(32), np.arange(32)] = 1.0

    den = FRs[None, :] ** 2 + c[:, None].astype(np.float64) ** 2
    den[0, 0] = 1.0
    t_u = c[:, None] / den                           # [32, 64]
    s_v = -FRs[None, :] / den

    # uncurl consts in transposed [kx, (j-rep 16, c 32)] layout, partition-
    # aligned with psT2 = [acv_I (rows 0-63); acv_R (rows 64-127)]:
    #   BTu = psT2 * [-t; +t],  BTv = psT2 * [-s; +s]
    tmat = np.tile(t_u.T[:, None, :], (1, 16, 1)).reshape(64, 512)
    smat = np.tile(s_v.T[:, None, :], (1, 16, 1)).reshape(64, 512)
    tTu = np.concatenate([-tmat, tmat], axis=0)      # [128, 512]
    tTv = np.concatenate([-smat, smat], axis=0)

    w_c = np.where(c == 0, 1.0, 2.0)
    s_q = 2.0 / (128.0 * 128.0)
    QYc = s_q * w_c[:, None] * np.cos(2 * np.pi * np.outer(c, Y) / 128)   # [32, 128]
    QYs = -s_q * w_c[:, None] * np.sin(2 * np.pi * np.outer(c, Y) / 128)
    Z32 = np.zeros((32, 128))
    # K=64 zero-padded Y-stage consts (K=32 stationary matmuls fault on HW):
    # top variants contract the first 32 K rows (j even), bottom the last 32;
    # replicated x2 down partitions for base-0 / base-64 slicing
    QYcT = np.tile(np.concatenate([QYc, Z32], axis=0), (2, 1))   # [128, 128]
    QYcB = np.tile(np.concatenate([Z32, QYc], axis=0), (2, 1))
    QYsT = np.tile(np.concatenate([QYs, Z32], axis=0), (2, 1))
    QYsB = np.tile(np.concatenate([Z32, QYs], axis=0), (2, 1))

    PRT = np.cos(2 * np.pi * np.outer(FRs, X) / 128)   # [r=64, X=128]
    PIT = np.sin(2 * np.pi * np.outer(FRs, X) / 128)
    PRT[32, :] = 0.0
    PIT[32, :] = 0.0
    # X-first synthesis const [128=(kxR,kxI), 256=(rg 2, X)]
    PPbig = np.zeros((128, 256))
    PPbig[0:64, 0:128] = PRT
    PPbig[64:128, 0:128] = -PIT
    PPbig[0:64, 128:256] = PIT
    PPbig[64:128, 128:256] = PRT

    # direct fr path: fr_i = Rx @ f_i @ Cy^T (pure 2x Fourier upsampling)
    ExRm = np.cos(2 * np.pi * np.outer(kx, x) / 64)
    ExIm = -np.sin(2 * np.pi * np.outer(kx, x) / 64)
    EyRm = np.cos(2 * np.pi * np.outer(c, y) / 64)
    EyIm = -np.sin(2 * np.pi * np.outer(c, y) / 64)
    QRm = s_q * w_c[None, :] * np.cos(2 * np.pi * np.outer(Y, c) / 128)
    QIm = s_q * w_c[None, :] * np.sin(2 * np.pi * np.outer(Y, c) / 128)
    Rx = PRT.T @ ExRm - PIT.T @ ExIm                 # [128, 64] (PRT.T == PR)
    Cy = QRm @ EyRm - QIm @ EyIm                     # [128, 64]
    RxT = Rx.T                                       # [x=64, X=128]
    CyT = np.concatenate([Cy.T, Cy.T], axis=0)       # [128, 128] doubled rows

    ImI = np.concatenate([np.eye(128), -np.eye(128)], axis=1)  # [128, 256]

    bf = lambda a: np.ascontiguousarray(a, dtype=ml_dtypes.bfloat16)
    f32 = lambda a: np.ascontiguousarray(a, dtype=np.float32)
    return dict(ExF=bf(ExF), EyCT=bf(EyCT), EyST=bf(EyST),
                S_sel=bf(S_sel), tTu=f32(tTu), tTv=f32(tTv),
                QYcT=bf(QYcT), QYcB=bf(QYcB), QYsT=bf(QYsT), QYsB=bf(QYsB),
                PPbig=bf(PPbig), RxT=bf(RxT), CyT=bf(CyT), ImI=bf(ImI))


def _rot90_kernel(k):
    # z[..., i, j] = k[..., (-j) mod n, i]
    y = np.swapaxes(k, -2, -1)
    return np.concatenate([y[..., :1], y[..., :0:-1]], axis=-1)


def _symmetric_kernel(k):
    k1 = k
    k2 = _rot90_kernel(k1)
    k3 = _rot90_kernel(k2)
    k4 = _rot90_kernel(k3)
    k5 = np.swapaxes(k1, -2, -1)
    k6 = _rot90_kernel(k5)
    k7 = _rot90_kernel(k6)
    k8 = _rot90_kernel(k7)
    return (k1 + k2 + k3 + k4 + k5 + k6 + k7 + k8) / 8.0


def _prep_k_all(kernel_np):
    """kernel [1,8,16,64,64] -> k_all [128, 2048] conv-layout packed (bf16)."""
    ksym = _symmetric_kernel(kernel_np.astype(np.float64))[0]   # [8,16,64,64]
    K = np.fft.rfft2(ksym).real                                  # [8,16,64,33]
    Kc = np.transpose(K[:, :, :, :32], (0, 1, 3, 2)).copy()      # [i,j,c,kx]
    Kc[:, :, :, 32] = 0.0                                        # kx nyquist
    k_all = np.zeros((128, 2048), dtype=np.float32)
    for i in range(8):
        h, im = i // 4, i % 4
        for j in range(16):
            k_all[im * 32:(im + 1) * 32, j * 128 + h * 64: j * 128 + h * 64 + 64] = Kc[i, j]
    return np.ascontiguousarray(k_all, dtype=ml_dtypes.bfloat16)


# ---------------------------------------------------------------------------
# device program
# ---------------------------------------------------------------------------

def _bcast(ap, n, axis_pos=1):
    """Insert a zero-step broadcast dim of size n into an AP (after partition dim)."""
    dims = list(ap.ap)
    dims.insert(axis_pos, [0, n])
    return AP(ap.tensor, ap.offset, dims)


def _view(ap, offset_elems, dims):
    """Raw AP view on the same tensor: explicit offset (elems) + [step, count] dims."""
    return AP(ap.tensor, ap.offset + offset_elems, dims)


def build_program(reps=1, ablate=(), gps_subs=False, gps_prod8=0, gps_conv=False,
                  sub_pool8=4, pe_sub8=0, **_unused):
    """ablate: subset of {'cross','synth','conv','dma'} to skip (profiling)."""
    nc = bacc.Bacc("TRN2", target_bir_lowering=False)
    consts = _host_consts()

    f_in = nc.dram_tensor("f_in", [B_PER_CORE, C1, 64, 64], F32, kind="ExternalInput")
    k_in = nc.dram_tensor("k_all", [128, 2048], BF16, kind="ExternalInput")
    # transposed output layout [b, X, ch, Y]; host returns .transpose(0,2,1,3) view
    out_sh = nc.dram_tensor("out_sh", [B_PER_CORE, 128, NCH_OUT, 128], F32,
                            kind="ExternalOutput")

    cdr = {name: nc.inline_tensor(arr, name=f"c_{name}") for name, arr in consts.items()}

    with tile.TileContext(nc) as tc:
        with (
            tc.tile_pool(name="cp", bufs=1) as cp,
            tc.tile_pool(name="fld", bufs=1) as fld,     # u_all/v_all/fr_all
            tc.tile_pool(name="wk", bufs=3) as wk,       # small working tiles
            tc.tile_pool(name="mw", bufs=2) as mwp,      # conv wide tiles
            tc.tile_pool(name="wp", bufs=3) as wp,       # cross product blocks
            tc.tile_pool(name="crp", bufs=3) as crp,     # cross output staging
            tc.tile_pool(name="pp", bufs=1, space="PSUM") as pp,
        ):
            # ---- load constants (stage1 deps first, spread over rings) ----
            cs = {}
            const_rings = [nc.scalar, nc.gpsimd]
            order = ['ExF', 'RxT', 'EyCT', 'EyST', 'S_sel', 'CyT',
                     'tTu', 'tTv', 'PPbig', 'QYcT', 'QYcB', 'QYsT', 'QYsB',
                     'ImI']
            for ci, name in enumerate(order):
                arr = consts[name]
                dt = BF16 if arr.dtype == ml_dtypes.bfloat16 else F32
                t = cp.tile(list(arr.shape), dt, tag=f"c_{name}", name=f"cs_{name}")
                const_rings[ci % 2].dma_start(out=t[:], in_=cdr[name][:])
                cs[name] = t
            k_sb = cp.tile([128, 2048], BF16, tag="k_sb")
            nc.gpsimd.dma_start(out=k_sb[:], in_=k_in[:])

            u_all = fld.tile([128, 16 * 256], BF16, tag="u_all")
            v_all = fld.tile([128, 16 * 256], BF16, tag="v_all")
            fr_all = fld.tile([128, 8 * 256], F32, tag="fr_all")

            dma_tick = [0]
            # weighted ring pattern: SP is otherwise idle, favor it
            ring_pats = {
                0: [nc.sync, nc.scalar, nc.sync, nc.gpsimd, nc.sync, nc.scalar],
                1: [nc.sync, nc.scalar, nc.gpsimd],
                2: [nc.sync, nc.scalar, nc.sync, nc.gpsimd],
                3: [nc.sync, nc.sync, nc.scalar, nc.sync, nc.sync, nc.gpsimd],
                4: [nc.sync, nc.gpsimd, nc.sync, nc.gpsimd, nc.sync, nc.scalar],
            }
            import os as _os
            out_rings = ring_pats[int(_os.environ.get("KRING", "0"))]

            def out_dma(out_ap, in_ap):
                eng = out_rings[dma_tick[0] % len(out_rings)]
                dma_tick[0] += 1
                eng.dma_start(out=out_ap, in_=in_ap)

            prod_tick = [0]

            def prod_eng():
                i = prod_tick[0] % 8
                prod_tick[0] += 1
                return nc.gpsimd if i < gps_prod8 else nc.vector

            sub_tick = [0]

            def sub_eng():
                i = sub_tick[0] % 8
                sub_tick[0] += 1
                return nc.gpsimd if i < sub_pool8 else nc.vector

            def emit_cross_block(gI, gJ, b):
                """cross products for channel groups gI x gJ, one sample.

                One fused [128, 2048] product op per W-block (16 pairs),
                one fused subtract + one 16-channel DMA per off-diag block."""
                W1 = wp.tile([128, 2048], BF16, tag="W1", name="W1")
                # W1[(a, bl, f)] = u[4gI+a] * v[4gJ+bl]
                in0 = _view(u_all[:], gI * 1024 + b * 128,
                            [u_all[:].ap[0], [256, 4], [0, 4], [1, 128]])
                in1 = _view(v_all[:], gJ * 1024 + b * 128,
                            [v_all[:].ap[0], [0, 4], [256, 4], [1, 128]])
                prod_eng().tensor_mul(
                    W1[:].rearrange("p (a c f) -> p a c f", a=4, c=4), in0, in1)
                if gI != gJ:
                    W2 = wp.tile([128, 2048], BF16, tag="W2", name="W2")
                    # W2[(bl, a, f)] = u[4gJ+bl] * v[4gI+a]
                    in0 = _view(u_all[:], gJ * 1024 + b * 128,
                                [u_all[:].ap[0], [256, 4], [0, 4], [1, 128]])
                    in1 = _view(v_all[:], gI * 1024 + b * 128,
                                [v_all[:].ap[0], [0, 4], [256, 4], [1, 128]])
                    prod_eng().tensor_mul(
                        W2[:].rearrange("p (c a f) -> p c a f", c=4, a=4), in0, in1)
                    # cr[(a, bl, f)] = W1[(a, bl, f)] - W2[(bl, a, f)]
                    cr = crp.tile([128, 2048], F32, tag="crb", name="crb")
                    st_i = sub_tick[0] % 8
                    sub_tick[0] += 1
                    if st_i < pe_sub8:
                        # subtract on the TensorEngine: psC = I@W1q - I@W2q',
                        # f32 chunk copies land on ACT
                        for q in range(4):
                            psC = pp.tile([128, 512], F32,
                                          tag=f"bankF{2 + q % 2}", bufs=1,
                                          name="psC")
                            w2q = _view(W2[:], q * 128,
                                        [W2[:].ap[0], [512, 4], [1, 128]])
                            nc.tensor.matmul(psC[:], cs["ImI"][:, 0:128],
                                             W1[:, q * 512:(q + 1) * 512],
                                             start=True, stop=False)
                            nc.tensor.matmul(
                                psC[:].rearrange("p (c f) -> p c f", c=4),
                                cs["ImI"][:, 128:256], w2q,
                                start=False, stop=True)
                            nc.scalar.copy(out=cr[:, q * 512:(q + 1) * 512],
                                           in_=psC[:])
                    else:
                        in1s = _view(W2[:], 0,
                                     [W2[:].ap[0], [128, 4], [512, 4], [1, 128]])
                        eng = nc.gpsimd if st_i < pe_sub8 + sub_pool8 else nc.vector
                        eng.tensor_sub(
                            cr[:].rearrange("p (a c f) -> p a c f", a=4, c=4),
                            W1[:].rearrange("p (a c f) -> p a c f", a=4, c=4), in1s)
                    # pair channels are contiguous per a only (stride 15-a
                    # between a rows) -> one 4-channel DMA per a
                    if 'dma' not in ablate:
                        for ai in range(4):
                            pch = 8 + _PAIR_IDX[(4 * gI + ai, 4 * gJ)]
                            out_dma(out_sh[b, :, pch:pch + 4, :],
                                    cr[:, ai * 512:(ai + 1) * 512].rearrange(
                                        "x (c y) -> x c y", c=4))
                else:
                    for ai in range(3):
                        a = 4 * gI + ai
                        cnt = 3 - ai
                        cr = crp.tile([128, 512], F32, tag="cr", name="cr")
                        in0 = _view(W1[:], ai * 512 + (ai + 1) * 128,
                                    [W1[:].ap[0], [128, cnt], [1, 128]])
                        in1 = _view(W1[:], (ai + 1) * 512 + ai * 128,
                                    [W1[:].ap[0], [512, cnt], [1, 128]])
                        sub_eng().tensor_sub(
                            cr[:, 0:cnt * 128].rearrange(
                                "p (cb f) -> p cb f", cb=cnt), in0, in1)
                        pch = 8 + _PAIR_IDX[(a, a + 1)]
                        if 'dma' not in ablate:
                            # small diag transfers: pin to the cheap SP ring
                            nc.sync.dma_start(
                                out=out_sh[b, :, pch:pch + cnt, :],
                                in_=cr[:, 0:cnt * 128].rearrange(
                                    "x (c y) -> x c y", c=cnt))

            def emit_stage1(b, st):
                A_ch = []
                T1s = []
                for ip in range(4):
                    fsb = wk.tile([64, 128], F32, tag="fsb", name="fsb")
                    nc.sync.dma_start(
                        out=fsb[:].rearrange("x (i y) -> x i y", i=2),
                        in_=f_in[b, 2 * ip:2 * ip + 2].rearrange("i x y -> x i y"))
                    fsb_bt = wk.tile([64, 128], BF16, tag="fsbb", name="fsbb")
                    nc.vector.tensor_copy(fsb_bt[:], fsb[:])
                    fsb_b = fsb_bt[:]
                    psA = pp.tile([128, 192], F32, tag="bankA", bufs=2, name="psA")
                    nc.tensor.matmul(psA[:], fsb_b, cs["ExF"][:], start=True, stop=True)
                    for iloc in range(2):
                        a_t = wk.tile([64, 192], BF16, tag=f"ach{2*ip+iloc}",
                                      name=f"ach{2*ip+iloc}")
                        nc.vector.tensor_copy(a_t[:], psA[iloc * 64:(iloc + 1) * 64, :])
                        A_ch.append(a_t)
                    # fr path: T1 = [f_i^T Rx^T | f_{i+1}^T Rx^T]  ([y, X] per channel)
                    psT1 = pp.tile([128, 128], F32, tag="bankA", bufs=2, name="psT1")
                    nc.tensor.matmul(psT1[:], fsb_b, cs["RxT"][:], start=True, stop=True)
                    t1sb = wk.tile([128, 128], BF16, tag=f"t1sb{ip}", name=f"t1sb{ip}")
                    nc.scalar.copy(out=t1sb[:], in_=psT1[:])
                    T1s.append(t1sb)
                st['A_ch'] = A_ch
                st['T1s'] = T1s

            def emit_stage2(b, st):
                A_ch = st['A_ch']
                # out free = [F_R(kx64) | F_I(kx64)] per tile
                psFcv = [pp.tile([128, 128], F32, tag=f"bankF{2+h}", name=f"psFcv{h}")
                         for h in range(2)]
                EyC, EyS = cs["EyCT"], cs["EyST"]
                for i in range(8):
                    A_RI = A_ch[i][:, 0:128]     # [A_R | A_I]
                    A_IS = A_ch[i][:, 64:192]    # [A_I | -A_R]
                    h, im = i // 4, i % 4
                    sl = slice(im * 32, (im + 1) * 32)
                    tp = (0, im * 32)
                    nc.tensor.matmul(psFcv[h][sl, :], EyC[0:64, :], A_RI,
                                     start=True, stop=False, tile_position=tp)
                    nc.tensor.matmul(psFcv[h][sl, :], EyS[0:64, :], A_IS,
                                     start=False, stop=True, tile_position=tp)

                Fcv = wk.tile([128, 256], BF16, tag="Fcv", name="Fcv")
                for h in range(2):
                    nc.scalar.copy(out=Fcv[:, h * 64:(h + 1) * 64], in_=psFcv[h][:, 0:64])
                    nc.scalar.copy(out=Fcv[:, 128 + h * 64:128 + (h + 1) * 64],
                                   in_=psFcv[h][:, 64:128])
                st['Fcv'] = Fcv

            def emit_conv(b, st):
                Fcv = st['Fcv']
                Mw = []
                for RI in range(2):
                    m_t = mwp.tile([128, 2048], BF16, tag=f"mw{RI}", name=f"mw{RI}")
                    in0 = _bcast(Fcv[:, RI * 128:(RI + 1) * 128], 16)
                    conv_eng = nc.gpsimd if gps_conv else nc.vector
                    conv_eng.tensor_mul(
                        m_t[:].rearrange("p (j f) -> p j f", j=16),
                        in0,
                        k_sb[:].rearrange("p (j f) -> p j f", j=16))
                    Mw.append(m_t)

                # selector-transpose: psT2 rows 0-63 = acv_I^T [kx, (j,c)],
                # rows 64-127 = acv_R^T (i-sum via stationary=Mw, moving=S_sel)
                psT2 = pp.tile([128, 512], F32, tag="bankT", bufs=1, name="psT2")
                for RI in range(2):
                    rows = slice(64, 128) if RI == 0 else slice(0, 64)
                    for j in range(16):
                        for h in range(2):
                            lhsT = _view(Mw[RI][:], j * 128 + h * 64,
                                         [Mw[RI][:].ap[0], [1, 64]])
                            nc.tensor.matmul(
                                psT2[rows, j * 32:(j + 1) * 32],
                                lhsT, cs["S_sel"][:],
                                start=(h == 0), stop=(h == 1))
                # uncurl in transposed layout: BT [128=(kxR,kxI), 512=(j,c)]
                BTu = wk.tile([128, 512], BF16, tag="BTu", name="BTu")
                BTv = wk.tile([128, 512], BF16, tag="BTv", name="BTv")
                nc.vector.tensor_mul(BTu[:], psT2[:], cs["tTu"][:])
                nc.vector.tensor_mul(BTv[:], psT2[:], cs["tTv"][:])
                st['BT'] = (BTu, BTv)

            def emit_fr(b, st):
                # fr direct: fr_i = (T1_i)^T @ Cy^T via one matmul per channel
                for i in range(8):
                    ip, iloc = i // 2, i % 2
                    t1 = st['T1s'][ip][iloc * 64:(iloc + 1) * 64, :]
                    psUf = pp.tile([128, 128], F32, tag="bankU", bufs=1, name="psUf")
                    nc.tensor.matmul(psUf[:], t1,
                                     cs["CyT"][iloc * 64:(iloc + 1) * 64, :],
                                     start=True, stop=True)
                    nc.scalar.copy(out=fr_all[:, i * 256 + b * 128:i * 256 + (b + 1) * 128],
                                   in_=psUf[:])
                if 'dma' not in ablate:
                    frv = _view(fr_all[:], b * 128,
                                [fr_all[:].ap[0], [256, 8], [1, 128]])
                    nc.sync.dma_start(out=out_sh[b, :, 0:8, :], in_=frv)

            uvcp_tick = [0]

            def emit_synth_group(b, st, g):
                """synthesize u and v channels 4g..4g+3 via X-first 2-stage DFT."""
                BTu, BTv = st['BT']
                for fi, (BT, dest) in enumerate(((BTu, u_all), (BTv, v_all))):
                    # X-stage: psHT [(4j, c), (rg, X)] = BT-slice^T @ PPbig
                    psHT = pp.tile([128, 256], F32, tag=f"bankF{fi}", name="psHT")
                    nc.tensor.matmul(psHT[:], BT[:, g * 128:(g + 1) * 128],
                                     cs["PPbig"][:], start=True, stop=True)
                    # PE stationary base partition must be 0/32/64: split rows
                    HTa = wk.tile([64, 256], BF16, tag=f"HTa{fi}", name=f"HTa{fi}")
                    HTb = wk.tile([64, 256], BF16, tag=f"HTb{fi}", name=f"HTb{fi}")
                    nc.scalar.copy(out=HTa[:], in_=psHT[0:64, :])
                    nc.scalar.copy(out=HTb[:], in_=psHT[64:128, :])
                    # Y-stage: psF[X, (4j, Y)] = sum_c H_R QYc + H_I QYs
                    # (K=64 zero-padded: K=32 stationary matmuls fault on HW)
                    psF = pp.tile([128, 512], F32, tag=f"bankF{2 + fi}", name="psF")
                    for jl in range(4):
                        HT = HTa if jl < 2 else HTb
                        qc = cs["QYcT"] if jl % 2 == 0 else cs["QYcB"]
                        qs = cs["QYsT"] if jl % 2 == 0 else cs["QYsB"]
                        osl = psF[:, jl * 128:(jl + 1) * 128]
                        nc.tensor.matmul(osl, HT[:, 0:128], qc[0:64, :],
                                         start=True, stop=False)
                        nc.tensor.matmul(osl, HT[:, 128:256], qs[0:64, :],
                                         start=False, stop=True)
                    dsl = _view(dest[:], (4 * g) * 256 + b * 128,
                                [dest[:].ap[0], [256, 4], [1, 128]])
                    nc.scalar.copy(out=dsl, in_=psF[:].rearrange(
                        "p (c y) -> p c y", c=4))
                    uvcp_tick[0] += 1

            for rep in range(reps):
                st = {b: {} for b in range(B_PER_CORE)}
                for b in range(B_PER_CORE):
                    emit_stage1(b, st[b])
                    emit_stage2(b, st[b])
                    if 'conv' in ablate:
                        continue
                    emit_conv(b, st[b])
                if 'conv' in ablate:
                    continue
                if 'synth' in ablate:
                    continue
                for b in range(B_PER_CORE):
                    emit_fr(b, st[b])
                for g in range(4):
                    for b in range(B_PER_CORE):
                        emit_synth_group(b, st[b], g)
                        if 'cross' in ablate:
                            continue
                        for gI in range(g + 1):
                            emit_cross_block(gI, g, b)
    nc.compile()
    return nc


# ---------------------------------------------------------------------------
# entry point
# ---------------------------------------------------------------------------

_PROGRAM = {}


def _get_program(reps=1, ablate=(), **kw):
    global _PROGRAM
    import os
    if 'gps_subs' not in kw:
        kw['gps_subs'] = os.environ.get("KGPS", "0") == "1"
    if 'gps_prod8' not in kw:
        kw['gps_prod8'] = int(os.environ.get("KGPSP", "0"))
    if 'gps_conv' not in kw:
        kw['gps_conv'] = os.environ.get("KGPSC", "0") == "1"
    if 'sub_pool8' not in kw:
        kw['sub_pool8'] = int(os.environ.get("KSUBP", "8"))
    if 'pe_sub8' not in kw:
        kw['pe_sub8'] = int(os.environ.get("KPESUB", "0"))
    key = (reps, tuple(sorted(ablate)), tuple(sorted(kw.items())))
    if key not in _PROGRAM:
        _PROGRAM[key] = build_program(reps, ablate=ablate, **kw)
    return _PROGRAM[key]


LAST_EXEC_NS = None
LAST_RESULT = None


def kernel(f, kernel):
    global LAST_EXEC_NS, LAST_RESULT
    f = np.ascontiguousarray(f, dtype=np.float32)
    k_all = _prep_k_all(np.asarray(kernel))
    nc = _get_program()
    in_maps = [
        {"f_in": f[2 * c:2 * c + 2], "k_all": k_all} for c in range(N_CORES)
    ]
    import os
    trace = bool(os.environ.get("KERNEL_TRACE"))
    res = run_bass_kernel_spmd(nc, in_maps, list(range(N_CORES)), trace=trace)
    LAST_RESULT = res
    if res.exec_time_ns is not None:
        LAST_EXEC_NS = res.exec_time_ns
    out = np.concatenate([res.results[c]["out_sh"] for c in range(N_CORES)], axis=0)
    # device layout is [b, X, ch, Y]; return the [b, ch, X, Y] view
    return out.transpose(0, 2, 1, 3)


# revision 100
# speedup vs baseline: 1.0451x; 1.0213x over previous
"""Trainium2 Bass kernel for nn_EquivariantLayer (spectral equivariant layer).

Strategy (data-parallel over batch, 2 samples/core x 8 cores):
  All FFTs are expressed as real DFT matmuls on the TensorEngine with layouts
  chosen so no corner-turn transposes are ever needed:

    stage1:  A = f^T @ [ExR^T | ExI^T]          (contract x; out [y, (RI,kx)])
    stage2:  F = Ey @ A                          (contract y; out [c, kx], c-major)
    conv:    M = F (*) K elementwise (K = rfft2(sym kernel) is REAL since the
             symmetrized kernel is D4-symmetric); the i-reduction runs as a
             "selector-transpose" matmul (stationary = M-slice, moving =
             selector) so the conv spectrum lands TRANSPOSED [kx, (j, c)]
    uncurl:  TO_U = i*t, TO_V = i*s are pure-imaginary -> one fused real mult
             per field over the partition-stacked [acv_I; acv_R] PSUM tile
    synth:   X-first 2-stage iDFT: psHT[(4j,c),(RI,X)] = BT^T @ PPbig, then
             psF[X,(4j,Y)] = sum_c H_R QYc + H_I QYs with K=64 zero-padded
             stationaries (K=32 stationary matmuls fault on TRN2 HW)
    cross:   u_a v_b - u_b v_a: fused [128,2048] bf16 products (16 pairs per
             DVE op via zero-step broadcast APs), f32 subtract on gpsimd/DVE

  The compute pipeline runs in bf16 (PE at 1 cycle/row, DVE 2x mode); PSUM
  accumulation stays fp32 and the final subtraction materializes fp32 output.
  Output DMAs round-robin over the SP / ACT / SWDGE rings (SP-weighted).
  HW constraints honored: gpsimd never touches PSUM; tensor_tensor operands
  share a partition base; PE stationary bases are 0/32/64 with K >= 64.

Output [16, 128, 128, 128] f32 (~134 MB) dominates traffic (memory regime).
"""
import sys
import numpy as np
import ml_dtypes

if '/opt/trn_rl_repo' not in sys.path:
    sys.path.insert(0, '/opt/trn_rl_repo')

import concourse.bass as bass
from concourse import bacc
import concourse.mybir as mybir
import concourse.tile as tile
from concourse.bass import AP
from concourse.bass_utils import run_bass_kernel_spmd

F32 = mybir.dt.float32
BF16 = mybir.dt.bfloat16
N_CORES = 8
B_PER_CORE = 2
C1, C2, N1, N2 = 8, 16, 64, 128
NCH_OUT = 128  # 8 fr + 120 cross

I_IDX, J_IDX = np.triu_indices(C2, 1)
_PAIR_IDX = {}
for _p, (_a, _b) in enumerate(zip(I_IDX, J_IDX)):
    _PAIR_IDX[(int(_a), int(_b))] = _p


# ---------------------------------------------------------------------------
# host-side constant construction
# ---------------------------------------------------------------------------

def _host_consts():
    x = np.arange(64)
    kx = np.arange(64)
    c = np.arange(32)
    y = np.arange(64)
    X = np.arange(128)
    Y = np.arange(128)

    FRs = np.where(kx <= 32, kx, kx - 64).astype(np.float64)  # signed row freq

    ExR = np.cos(2 * np.pi * np.outer(kx, x) / 64)   # [kx, x]
    ExI = -np.sin(2 * np.pi * np.outer(kx, x) / 64)
    # [A_R | A_I | -A_R] so stage2 fuses R/I into two matmuls
    ExF = np.concatenate([ExR.T, ExI.T, -ExR.T], axis=1)   # [x, 192]

    # F_R = C A_R + S A_I ; F_I = C A_I + S (-A_R)   (C=cos, S=sin)
    # replicated x2 down partitions so base-0 and base-64 slices both exist
    EyCT = np.tile(np.cos(2 * np.pi * np.outer(c, y) / 64).T, (2, 1))  # [128, 32]
    EyST = np.tile(np.sin(2 * np.pi * np.outer(c, y) / 64).T, (2, 1))

    S_sel = np.zeros((128, 32))
    for im in range(4):
        S_sel[im * 32 + np.arange(32), np.arange(32)] = 1.0

    den = FRs[None, :] ** 2 + c[:, None].astype(np.float64) ** 2
    den[0, 0] = 1.0
    t_u = c[:, None] / den                           # [32, 64]
    s_v = -FRs[None, :] / den

    # uncurl consts in transposed [kx, (j-rep 16, c 32)] layout, partition-
    # aligned with psT2 = [acv_I (rows 0-63); acv_R (rows 64-127)]:
    #   BTu = psT2 * [-t; +t],  BTv = psT2 * [-s; +s]
    tmat = np.tile(t_u.T[:, None, :], (1, 16, 1)).reshape(64, 512)
    smat = np.tile(s_v.T[:, None, :], (1, 16, 1)).reshape(64, 512)
    tTu = np.concatenate([-tmat, tmat], axis=0)      # [128, 512]
    tTv = np.concatenate([-smat, smat], axis=0)

    w_c = np.where(c == 0, 1.0, 2.0)
    s_q = 2.0 / (128.0 * 128.0)
    QYc = s_q * w_c[:, None] * np.cos(2 * np.pi * np.outer(c, Y) / 128)   # [32, 128]
    QYs = -s_q * w_c[:, None] * np.sin(2 * np.pi * np.outer(c, Y) / 128)
    Z32 = np.zeros((32, 128))
    # K=64 zero-padded Y-stage consts (K=32 stationary matmuls fault on HW):
    # top variants contract the first 32 K rows (j even), bottom the last 32;
    # replicated x2 down partitions for base-0 / base-64 slicing
    QYcT = np.tile(np.concatenate([QYc, Z32], axis=0), (2, 1))   # [128, 128]
    QYcB = np.tile(np.concatenate([Z32, QYc], axis=0), (2, 1))
    QYsT = np.tile(np.concatenate([QYs, Z32], axis=0), (2, 1))
    QYsB = np.tile(np.concatenate([Z32, QYs], axis=0), (2, 1))

    PRT = np.cos(2 * np.pi * np.outer(FRs, X) / 128)   # [r=64, X=128]
    PIT = np.sin(2 * np.pi * np.outer(FRs, X) / 128)
    PRT[32, :] = 0.0
    PIT[32, :] = 0.0
    # X-first synthesis const [128=(kxR,kxI), 256=(rg 2, X)]
    PPbig = np.zeros((128, 256))
    PPbig[0:64, 0:128] = PRT
    PPbig[64:128, 0:128] = -PIT
    PPbig[0:64, 128:256] = PIT
    PPbig[64:128, 128:256] = PRT

    # direct fr path: fr_i = Rx @ f_i @ Cy^T (pure 2x Fourier upsampling)
    ExRm = np.cos(2 * np.pi * np.outer(kx, x) / 64)
    ExIm = -np.sin(2 * np.pi * np.outer(kx, x) / 64)
    EyRm = np.cos(2 * np.pi * np.outer(c, y) / 64)
    EyIm = -np.sin(2 * np.pi * np.outer(c, y) / 64)
    QRm = s_q * w_c[None, :] * np.cos(2 * np.pi * np.outer(Y, c) / 128)
    QIm = s_q * w_c[None, :] * np.sin(2 * np.pi * np.outer(Y, c) / 128)
    Rx = PRT.T @ ExRm - PIT.T @ ExIm                 # [128, 64] (PRT.T == PR)
    Cy = QRm @ EyRm - QIm @ EyIm                     # [128, 64]
    RxT = Rx.T                                       # [x=64, X=128]
    CyT = np.concatenate([Cy.T, Cy.T], axis=0)       # [128, 128] doubled rows

    ImI = np.concatenate([np.eye(128), -np.eye(128)], axis=1)  # [128, 256]

    bf = lambda a: np.ascontiguousarray(a, dtype=ml_dtypes.bfloat16)
    f32 = lambda a: np.ascontiguousarray(a, dtype=np.float32)
    return dict(ExF=bf(ExF), EyCT=bf(EyCT), EyST=bf(EyST),
                S_sel=bf(S_sel), tTu=f32(tTu), tTv=f32(tTv),
                QYcT=bf(QYcT), QYcB=bf(QYcB), QYsT=bf(QYsT), QYsB=bf(QYsB),
                PPbig=bf(PPbig), RxT=bf(RxT), CyT=bf(CyT), ImI=bf(ImI))


def _rot90_kernel(k):
    # z[..., i, j] = k[..., (-j) mod n, i]
    y = np.swapaxes(k, -2, -1)
    return np.concatenate([y[..., :1], y[..., :0:-1]], axis=-1)


def _symmetric_kernel(k):
    k1 = k
    k2 = _rot90_kernel(k1)
    k3 = _rot90_kernel(k2)
    k4 = _rot90_kernel(k3)
    k5 = np.swapaxes(k1, -2, -1)
    k6 = _rot90_kernel(k5)
    k7 = _rot90_kernel(k6)
    k8 = _rot90_kernel(k7)
    return (k1 + k2 + k3 + k4 + k5 + k6 + k7 + k8) / 8.0


def _prep_k_all(kernel_np):
    """kernel [1,8,16,64,64] -> k_all [128, 2048] conv-layout packed (bf16)."""
    ksym = _symmetric_kernel(kernel_np.astype(np.float64))[0]   # [8,16,64,64]
    K = np.fft.rfft2(ksym).real                                  # [8,16,64,33]
    Kc = np.transpose(K[:, :, :, :32], (0, 1, 3, 2)).copy()      # [i,j,c,kx]
    Kc[:, :, :, 32] = 0.0                                        # kx nyquist
    k_all = np.zeros((128, 2048), dtype=np.float32)
    for i in range(8):
        h, im = i // 4, i % 4
        for j in range(16):
            k_all[im * 32:(im + 1) * 32, j * 128 + h * 64: j * 128 + h * 64 + 64] = Kc[i, j]
    return np.ascontiguousarray(k_all, dtype=ml_dtypes.bfloat16)


# ---------------------------------------------------------------------------
# device program
# ---------------------------------------------------------------------------

def _bcast(ap, n, axis_pos=1):
    """Insert a zero-step broadcast dim of size n into an AP (after partition dim)."""
    dims = list(ap.ap)
    dims.insert(axis_pos, [0, n])
    return AP(ap.tensor, ap.offset, dims)


def _view(ap, offset_elems, dims):
    """Raw AP view on the same tensor: explicit offset (elems) + [step, count] dims."""
    return AP(ap.tensor, ap.offset + offset_elems, dims)


def build_program(reps=1, ablate=(), gps_subs=False, gps_prod8=0, gps_conv=False,
                  sub_pool8=4, pe_sub8=0, **_unused):
    """ablate: subset of {'cross','synth','conv','dma'} to skip (profiling)."""
    nc = bacc.Bacc("TRN2", target_bir_lowering=False)
    consts = _host_consts()

    f_in = nc.dram_tensor("f_in", [B_PER_CORE, C1, 64, 64], F32, kind="ExternalInput")
    k_in = nc.dram_tensor("k_all", [128, 2048], BF16, kind="ExternalInput")
    # transposed output layout [b, X, ch, Y]; host returns .transpose(0,2,1,3) view
    out_sh = nc.dram_tensor("out_sh", [B_PER_CORE, 128, NCH_OUT, 128], F32,
                            kind="ExternalOutput")

    cdr = {name: nc.inline_tensor(arr, name=f"c_{name}") for name, arr in consts.items()}

    with tile.TileContext(nc) as tc:
        with (
            tc.tile_pool(name="cp", bufs=1) as cp,
            tc.tile_pool(name="fld", bufs=1) as fld,     # u_all/v_all/fr_all
            tc.tile_pool(name="wk", bufs=3) as wk,       # small working tiles
            tc.tile_pool(name="mw", bufs=2) as mwp,      # conv wide tiles
            tc.tile_pool(name="wp", bufs=3) as wp,       # cross product blocks
            tc.tile_pool(name="crp", bufs=3) as crp,     # cross output staging
            tc.tile_pool(name="pp", bufs=1, space="PSUM") as pp,
        ):
            # ---- load constants (stage1 deps first, spread over rings) ----
            cs = {}
            const_rings = [nc.scalar, nc.gpsimd]
            order = ['ExF', 'RxT', 'EyCT', 'EyST', 'S_sel', 'CyT',
                     'tTu', 'tTv', 'PPbig', 'QYcT', 'QYcB', 'QYsT', 'QYsB',
                     'ImI']
            for ci, name in enumerate(order):
                arr = consts[name]
                dt = BF16 if arr.dtype == ml_dtypes.bfloat16 else F32
                t = cp.tile(list(arr.shape), dt, tag=f"c_{name}", name=f"cs_{name}")
                const_rings[ci % 2].dma_start(out=t[:], in_=cdr[name][:])
                cs[name] = t
            k_sb = cp.tile([128, 2048], BF16, tag="k_sb")
            nc.gpsimd.dma_start(out=k_sb[:], in_=k_in[:])

            u_all = fld.tile([128, 16 * 256], BF16, tag="u_all")
            v_all = fld.tile([128, 16 * 256], BF16, tag="v_all")
            fr_all = fld.tile([128, 8 * 256], F32, tag="fr_all")

            dma_tick = [0]
            # weighted ring pattern: SP is otherwise idle, favor it
            ring_pats = {
                0: [nc.sync, nc.scalar, nc.sync, nc.gpsimd, nc.sync, nc.scalar],
                1: [nc.sync, nc.scalar, nc.gpsimd],
                2: [nc.sync, nc.scalar, nc.sync, nc.gpsimd],
                3: [nc.sync, nc.sync, nc.scalar, nc.sync, nc.sync, nc.gpsimd],
                4: [nc.sync, nc.gpsimd, nc.sync, nc.gpsimd, nc.sync, nc.scalar],
            }
            import os as _os
            out_rings = ring_pats[int(_os.environ.get("KRING", "0"))]

            def out_dma(out_ap, in_ap):
                eng = out_rings[dma_tick[0] % len(out_rings)]
                dma_tick[0] += 1
                eng.dma_start(out=out_ap, in_=in_ap)

            prod_tick = [0]

            def prod_eng():
                i = prod_tick[0] % 8
                prod_tick[0] += 1
                return nc.gpsimd if i < gps_prod8 else nc.vector

            sub_tick = [0]

            def sub_eng():
                i = sub_tick[0] % 8
                sub_tick[0] += 1
                return nc.gpsimd if i < sub_pool8 else nc.vector

            def emit_cross_block(gI, gJ, b):
                """cross products for channel groups gI x gJ, one sample.

                One fused [128, 2048] product op per W-block (16 pairs),
                one fused subtract + one 16-channel DMA per off-diag block."""
                W1 = wp.tile([128, 2048], BF16, tag="W1", name="W1")
                # W1[(a, bl, f)] = u[4gI+a] * v[4gJ+bl]
                in0 = _view(u_all[:], gI * 1024 + b * 128,
                            [u_all[:].ap[0], [256, 4], [0, 4], [1, 128]])
                in1 = _view(v_all[:], gJ * 1024 + b * 128,
                            [v_all[:].ap[0], [0, 4], [256, 4], [1, 128]])
                prod_eng().tensor_mul(
                    W1[:].rearrange("p (a c f) -> p a c f", a=4, c=4), in0, in1)
                if gI != gJ:
                    W2 = wp.tile([128, 2048], BF16, tag="W2", name="W2")
                    # W2[(bl, a, f)] = u[4gJ+bl] * v[4gI+a]
                    in0 = _view(u_all[:], gJ * 1024 + b * 128,
                                [u_all[:].ap[0], [256, 4], [0, 4], [1, 128]])
                    in1 = _view(v_all[:], gI * 1024 + b * 128,
                                [v_all[:].ap[0], [0, 4], [256, 4], [1, 128]])
                    prod_eng().tensor_mul(
                        W2[:].rearrange("p (c a f) -> p c a f", c=4, a=4), in0, in1)
                    # cr[(a, bl, f)] = W1[(a, bl, f)] - W2[(bl, a, f)]
                    cr = crp.tile([128, 2048], F32, tag="crb", name="crb")
                    st_i = sub_tick[0] % 8
                    sub_tick[0] += 1
                    if st_i < pe_sub8:
                        # subtract on the TensorEngine: psC = I@W1q - I@W2q',
                        # f32 chunk copies land on ACT
                        for q in range(4):
                            psC = pp.tile([128, 512], F32,
                                          tag=f"bankF{2 + q % 2}", bufs=1,
                                          name="psC")
                            w2q = _view(W2[:], q * 128,
                                        [W2[:].ap[0], [512, 4], [1, 128]])
                            nc.tensor.matmul(psC[:], cs["ImI"][:, 0:128],
                                             W1[:, q * 512:(q + 1) * 512],
                                             start=True, stop=False)
                            nc.tensor.matmul(
                                psC[:].rearrange("p (c f) -> p c f", c=4),
                                cs["ImI"][:, 128:256], w2q,
                                start=False, stop=True)
                            nc.scalar.copy(out=cr[:, q * 512:(q + 1) * 512],
                                           in_=psC[:])
                    else:
                        in1s = _view(W2[:], 0,
                                     [W2[:].ap[0], [128, 4], [512, 4], [1, 128]])
                        eng = nc.gpsimd if st_i < pe_sub8 + sub_pool8 else nc.vector
                        eng.tensor_sub(
                            cr[:].rearrange("p (a c f) -> p a c f", a=4, c=4),
                            W1[:].rearrange("p (a c f) -> p a c f", a=4, c=4), in1s)
                    # pair channels are contiguous per a only (stride 15-a
                    # between a rows) -> one 4-channel DMA per a
                    if 'dma' not in ablate:
                        for ai in range(4):
                            pch = 8 + _PAIR_IDX[(4 * gI + ai, 4 * gJ)]
                            out_dma(out_sh[b, :, pch:pch + 4, :],
                                    cr[:, ai * 512:(ai + 1) * 512].rearrange(
                                        "x (c y) -> x c y", c=4))
                else:
                    for ai in range(3):
                        a = 4 * gI + ai
                        cnt = 3 - ai
                        cr = crp.tile([128, 512], F32, tag="cr", name="cr")
                        in0 = _view(W1[:], ai * 512 + (ai + 1) * 128,
                                    [W1[:].ap[0], [128, cnt], [1, 128]])
                        in1 = _view(W1[:], (ai + 1) * 512 + ai * 128,
                                    [W1[:].ap[0], [512, cnt], [1, 128]])
                        sub_eng().tensor_sub(
                            cr[:, 0:cnt * 128].rearrange(
                                "p (cb f) -> p cb f", cb=cnt), in0, in1)
                        pch = 8 + _PAIR_IDX[(a, a + 1)]
                        if 'dma' not in ablate:
                            # small diag transfers: pin to the cheap SP ring
                            nc.sync.dma_start(
                                out=out_sh[b, :, pch:pch + cnt, :],
                                in_=cr[:, 0:cnt * 128].rearrange(
                                    "x (c y) -> x c y", c=cnt))

            def emit_stage1(b, st):
                A_ch = []
                T1s = []
                for ip in range(4):
                    fsb = wk.tile([64, 128], F32, tag="fsb", name="fsb")
                    nc.sync.dma_start(
                        out=fsb[:].rearrange("x (i y) -> x i y", i=2),
                        in_=f_in[b, 2 * ip:2 * ip + 2].rearrange("i x y -> x i y"))
                    fsb_bt = wk.tile([64, 128], BF16, tag="fsbb", name="fsbb")
                    nc.vector.tensor_copy(fsb_bt[:], fsb[:])
                    fsb_b = fsb_bt[:]
                    psA = pp.tile([128, 192], F32, tag="bankA", bufs=2, name="psA")
                    nc.tensor.matmul(psA[:], fsb_b, cs["ExF"][:], start=True, stop=True)
                    for iloc in range(2):
                        a_t = wk.tile([64, 192], BF16, tag=f"ach{2*ip+iloc}",
                                      name=f"ach{2*ip+iloc}")
                        nc.vector.tensor_copy(a_t[:], psA[iloc * 64:(iloc + 1) * 64, :])
                        A_ch.append(a_t)
                    # fr path: T1 = [f_i^T Rx^T | f_{i+1}^T Rx^T]  ([y, X] per channel)
                    psT1 = pp.tile([128, 128], F32, tag="bankA", bufs=2, name="psT1")
                    nc.tensor.matmul(psT1[:], fsb_b, cs["RxT"][:], start=True, stop=True)
                    t1sb = wk.tile([128, 128], BF16, tag=f"t1sb{ip}", name=f"t1sb{ip}")
                    if b == 0:
                        nc.vector.tensor_copy(t1sb[:], psT1[:])
                    else:
                        nc.scalar.copy(out=t1sb[:], in_=psT1[:])
                    T1s.append(t1sb)
                st['A_ch'] = A_ch
                st['T1s'] = T1s

            def emit_stage2(b, st):
                A_ch = st['A_ch']
                # out free = [F_R(kx64) | F_I(kx64)] per tile
                psFcv = [pp.tile([128, 128], F32, tag=f"bankF{2+h}", name=f"psFcv{h}")
                         for h in range(2)]
                EyC, EyS = cs["EyCT"], cs["EyST"]
                for i in range(8):
                    A_RI = A_ch[i][:, 0:128]     # [A_R | A_I]
                    A_IS = A_ch[i][:, 64:192]    # [A_I | -A_R]
                    h, im = i // 4, i % 4
                    sl = slice(im * 32, (im + 1) * 32)
                    tp = (0, im * 32)
                    nc.tensor.matmul(psFcv[h][sl, :], EyC[0:64, :], A_RI,
                                     start=True, stop=False, tile_position=tp)
                    nc.tensor.matmul(psFcv[h][sl, :], EyS[0:64, :], A_IS,
                                     start=False, stop=True, tile_position=tp)

                Fcv = wk.tile([128, 256], BF16, tag="Fcv", name="Fcv")
                for h in range(2):
                    if b == 0:
                        nc.vector.tensor_copy(Fcv[:, h * 64:(h + 1) * 64],
                                              psFcv[h][:, 0:64])
                        nc.vector.tensor_copy(Fcv[:, 128 + h * 64:128 + (h + 1) * 64],
                                              psFcv[h][:, 64:128])
                    else:
                        nc.scalar.copy(out=Fcv[:, h * 64:(h + 1) * 64],
                                       in_=psFcv[h][:, 0:64])
                        nc.scalar.copy(out=Fcv[:, 128 + h * 64:128 + (h + 1) * 64],
                                       in_=psFcv[h][:, 64:128])
                st['Fcv'] = Fcv

            def emit_conv(b, st):
                Fcv = st['Fcv']
                Mw = []
                for RI in range(2):
                    m_t = mwp.tile([128, 2048], BF16, tag=f"mw{RI}", name=f"mw{RI}")
                    in0 = _bcast(Fcv[:, RI * 128:(RI + 1) * 128], 16)
                    conv_eng = nc.gpsimd if (gps_conv or b == 0) else nc.vector
                    conv_eng.tensor_mul(
                        m_t[:].rearrange("p (j f) -> p j f", j=16),
                        in0,
                        k_sb[:].rearrange("p (j f) -> p j f", j=16))
                    Mw.append(m_t)

                # selector-transpose: psT2 rows 0-63 = acv_I^T [kx, (j,c)],
                # rows 64-127 = acv_R^T (i-sum via stationary=Mw, moving=S_sel)
                psT2 = pp.tile([128, 512], F32, tag="bankT", bufs=1, name="psT2")
                for RI in range(2):
                    rows = slice(64, 128) if RI == 0 else slice(0, 64)
                    for j in range(16):
                        for h in range(2):
                            lhsT = _view(Mw[RI][:], j * 128 + h * 64,
                                         [Mw[RI][:].ap[0], [1, 64]])
                            nc.tensor.matmul(
                                psT2[rows, j * 32:(j + 1) * 32],
                                lhsT, cs["S_sel"][:],
                                start=(h == 0), stop=(h == 1))
                # uncurl in transposed layout: BT [128=(kxR,kxI), 512=(j,c)]
                BTu = wk.tile([128, 512], BF16, tag="BTu", name="BTu")
                BTv = wk.tile([128, 512], BF16, tag="BTv", name="BTv")
                nc.vector.tensor_mul(BTu[:], psT2[:], cs["tTu"][:])
                nc.vector.tensor_mul(BTv[:], psT2[:], cs["tTv"][:])
                st['BT'] = (BTu, BTv)

            def emit_fr(b, st):
                # fr direct: fr_i = (T1_i)^T @ Cy^T via one matmul per channel
                for i in range(8):
                    ip, iloc = i // 2, i % 2
                    t1 = st['T1s'][ip][iloc * 64:(iloc + 1) * 64, :]
                    psUf = pp.tile([128, 128], F32, tag="bankU", bufs=1, name="psUf")
                    nc.tensor.matmul(psUf[:], t1,
                                     cs["CyT"][iloc * 64:(iloc + 1) * 64, :],
                                     start=True, stop=True)
                    nc.scalar.copy(out=fr_all[:, i * 256 + b * 128:i * 256 + (b + 1) * 128],
                                   in_=psUf[:])
                if 'dma' not in ablate:
                    frv = _view(fr_all[:], b * 128,
                                [fr_all[:].ap[0], [256, 8], [1, 128]])
                    nc.sync.dma_start(out=out_sh[b, :, 0:8, :], in_=frv)

            uvcp_tick = [0]

            def emit_synth_group(b, st, g):
                """synthesize u and v channels 4g..4g+3 via X-first 2-stage DFT."""
                BTu, BTv = st['BT']
                for fi, (BT, dest) in enumerate(((BTu, u_all), (BTv, v_all))):
                    # X-stage: psHT [(4j, c), (rg, X)] = BT-slice^T @ PPbig
                    psHT = pp.tile([128, 256], F32, tag=f"bankF{fi}", name="psHT")
                    nc.tensor.matmul(psHT[:], BT[:, g * 128:(g + 1) * 128],
                                     cs["PPbig"][:], start=True, stop=True)
                    # PE stationary base partition must be 0/32/64: split rows
                    HTa = wk.tile([64, 256], BF16, tag=f"HTa{fi}", name=f"HTa{fi}")
                    HTb = wk.tile([64, 256], BF16, tag=f"HTb{fi}", name=f"HTb{fi}")
                    nc.scalar.copy(out=HTa[:], in_=psHT[0:64, :])
                    nc.scalar.copy(out=HTb[:], in_=psHT[64:128, :])
                    # Y-stage: psF[X, (4j, Y)] = sum_c H_R QYc + H_I QYs
                    # (K=64 zero-padded: K=32 stationary matmuls fault on HW)
                    psF = pp.tile([128, 512], F32, tag=f"bankF{2 + fi}", name="psF")
                    for jl in range(4):
                        HT = HTa if jl < 2 else HTb
                        qc = cs["QYcT"] if jl % 2 == 0 else cs["QYcB"]
                        qs = cs["QYsT"] if jl % 2 == 0 else cs["QYsB"]
                        osl = psF[:, jl * 128:(jl + 1) * 128]
                        nc.tensor.matmul(osl, HT[:, 0:128], qc[0:64, :],
                                         start=True, stop=False)
                        nc.tensor.matmul(osl, HT[:, 128:256], qs[0:64, :],
                                         start=False, stop=True)
                    dsl = _view(dest[:], (4 * g) * 256 + b * 128,
                                [dest[:].ap[0], [256, 4], [1, 128]])
                    nc.scalar.copy(out=dsl, in_=psF[:].rearrange(
                        "p (c y) -> p c y", c=4))
                    uvcp_tick[0] += 1

            for rep in range(reps):
                st = {b: {} for b in range(B_PER_CORE)}
                for b in range(B_PER_CORE):
                    emit_stage1(b, st[b])
                    emit_stage2(b, st[b])
                    if 'conv' in ablate:
                        continue
                    emit_conv(b, st[b])
                if 'conv' in ablate:
                    continue
                if 'synth' in ablate:
                    continue
                for b in range(B_PER_CORE):
                    emit_fr(b, st[b])
                for g in range(4):
                    for b in range(B_PER_CORE):
                        emit_synth_group(b, st[b], g)
                        if 'cross' in ablate:
                            continue
                        for gI in range(g + 1):
                            emit_cross_block(gI, g, b)
    nc.compile()
    return nc


# ---------------------------------------------------------------------------
# entry point
# ---------------------------------------------------------------------------

_PROGRAM = {}


def _get_program(reps=1, ablate=(), **kw):
    global _PROGRAM
    import os
    if 'gps_subs' not in kw:
        kw['gps_subs'] = os.environ.get("KGPS", "0") == "1"
    if 'gps_prod8' not in kw:
        kw['gps_prod8'] = int(os.environ.get("KGPSP", "0"))
    if 'gps_conv' not in kw:
        kw['gps_conv'] = os.environ.get("KGPSC", "0") == "1"
    if 'sub_pool8' not in kw:
        kw['sub_pool8'] = int(os.environ.get("KSUBP", "8"))
    if 'pe_sub8' not in kw:
        kw['pe_sub8'] = int(os.environ.get("KPESUB", "0"))
    key = (reps, tuple(sorted(ablate)), tuple(sorted(kw.items())))
    if key not in _PROGRAM:
        _PROGRAM[key] = build_program(reps, ablate=ablate, **kw)
    return _PROGRAM[key]


LAST_EXEC_NS = None
LAST_RESULT = None


def kernel(f, kernel):
    global LAST_EXEC_NS, LAST_RESULT
    f = np.ascontiguousarray(f, dtype=np.float32)
    k_all = _prep_k_all(np.asarray(kernel))
    nc = _get_program()
    in_maps = [
        {"f_in": f[2 * c:2 * c + 2], "k_all": k_all} for c in range(N_CORES)
    ]
    import os
    trace = bool(os.environ.get("KERNEL_TRACE"))
    res = run_bass_kernel_spmd(nc, in_maps, list(range(N_CORES)), trace=trace)
    LAST_RESULT = res
    if res.exec_time_ns is not None:
        LAST_EXEC_NS = res.exec_time_ns
    out = np.concatenate([res.results[c]["out_sh"] for c in range(N_CORES)], axis=0)
    # device layout is [b, X, ch, Y]; return the [b, ch, X, Y] view
    return out.transpose(0, 2, 1, 3)


# revision 102
# speedup vs baseline: 1.0481x; 1.0029x over previous
"""Trainium2 Bass kernel for nn_EquivariantLayer (spectral equivariant layer).

Strategy (data-parallel over batch, 2 samples/core x 8 cores):
  All FFTs are expressed as real DFT matmuls on the TensorEngine with layouts
  chosen so no corner-turn transposes are ever needed:

    stage1:  A = f^T @ [ExR^T | ExI^T]          (contract x; out [y, (RI,kx)])
    stage2:  F = Ey @ A                          (contract y; out [c, kx], c-major)
    conv:    M = F (*) K elementwise (K = rfft2(sym kernel) is REAL since the
             symmetrized kernel is D4-symmetric); the i-reduction runs as a
             "selector-transpose" matmul (stationary = M-slice, moving =
             selector) so the conv spectrum lands TRANSPOSED [kx, (j, c)]
    uncurl:  TO_U = i*t, TO_V = i*s are pure-imaginary -> one fused real mult
             per field over the partition-stacked [acv_I; acv_R] PSUM tile
    synth:   X-first 2-stage iDFT: psHT[(4j,c),(RI,X)] = BT^T @ PPbig, then
             psF[X,(4j,Y)] = sum_c H_R QYc + H_I QYs with K=64 zero-padded
             stationaries (K=32 stationary matmuls fault on TRN2 HW)
    cross:   u_a v_b - u_b v_a: fused [128,2048] bf16 products (16 pairs per
             DVE op via zero-step broadcast APs), f32 subtract on gpsimd/DVE

  The compute pipeline runs in bf16 (PE at 1 cycle/row, DVE 2x mode); PSUM
  accumulation stays fp32 and the final subtraction materializes fp32 output.
  Output DMAs round-robin over the SP / ACT / SWDGE rings (SP-weighted).
  HW constraints honored: gpsimd never touches PSUM; tensor_tensor operands
  share a partition base; PE stationary bases are 0/32/64 with K >= 64.

Output [16, 128, 128, 128] f32 (~134 MB) dominates traffic (memory regime).
"""
import sys
import numpy as np
import ml_dtypes

if '/opt/trn_rl_repo' not in sys.path:
    sys.path.insert(0, '/opt/trn_rl_repo')

import concourse.bass as bass
from concourse import bacc
import concourse.mybir as mybir
import concourse.tile as tile
from concourse.bass import AP
from concourse.bass_utils import run_bass_kernel_spmd

F32 = mybir.dt.float32
BF16 = mybir.dt.bfloat16
N_CORES = 8
B_PER_CORE = 2
C1, C2, N1, N2 = 8, 16, 64, 128
NCH_OUT = 128  # 8 fr + 120 cross

I_IDX, J_IDX = np.triu_indices(C2, 1)
_PAIR_IDX = {}
for _p, (_a, _b) in enumerate(zip(I_IDX, J_IDX)):
    _PAIR_IDX[(int(_a), int(_b))] = _p


# ---------------------------------------------------------------------------
# host-side constant construction
# ---------------------------------------------------------------------------

def _host_consts():
    x = np.arange(64)
    kx = np.arange(64)
    c = np.arange(32)
    y = np.arange(64)
    X = np.arange(128)
    Y = np.arange(128)

    FRs = np.where(kx <= 32, kx, kx - 64).astype(np.float64)  # signed row freq

    ExR = np.cos(2 * np.pi * np.outer(kx, x) / 64)   # [kx, x]
    ExI = -np.sin(2 * np.pi * np.outer(kx, x) / 64)
    # [A_R | A_I | -A_R] so stage2 fuses R/I into two matmuls
    ExF = np.concatenate([ExR.T, ExI.T, -ExR.T], axis=1)   # [x, 192]

    # F_R = C A_R + S A_I ; F_I = C A_I + S (-A_R)   (C=cos, S=sin)
    # replicated x2 down partitions so base-0 and base-64 slices both exist
    EyCT = np.tile(np.cos(2 * np.pi * np.outer(c, y) / 64).T, (2, 1))  # [128, 32]
    EyST = np.tile(np.sin(2 * np.pi * np.outer(c, y) / 64).T, (2, 1))

    S_sel = np.zeros((128, 32))
    for im in range(4):
        S_sel[im * 32 + np.arange(32), np.arange(32)] = 1.0

    den = FRs[None, :] ** 2 + c[:, None].astype(np.float64) ** 2
    den[0, 0] = 1.0
    t_u = c[:, None] / den                           # [32, 64]
    s_v = -FRs[None, :] / den

    # uncurl consts in transposed [kx, (j-rep 16, c 32)] layout, partition-
    # aligned with psT2 = [acv_I (rows 0-63); acv_R (rows 64-127)]:
    #   BTu = psT2 * [-t; +t],  BTv = psT2 * [-s; +s]
    tmat = np.tile(t_u.T[:, None, :], (1, 16, 1)).reshape(64, 512)
    smat = np.tile(s_v.T[:, None, :], (1, 16, 1)).reshape(64, 512)
    tTu = np.concatenate([-tmat, tmat], axis=0)      # [128, 512]
    tTv = np.concatenate([-smat, smat], axis=0)

    w_c = np.where(c == 0, 1.0, 2.0)
    s_q = 2.0 / (128.0 * 128.0)
    QYc = s_q * w_c[:, None] * np.cos(2 * np.pi * np.outer(c, Y) / 128)   # [32, 128]
    QYs = -s_q * w_c[:, None] * np.sin(2 * np.pi * np.outer(c, Y) / 128)
    Z32 = np.zeros((32, 128))
    # K=64 zero-padded Y-stage consts (K=32 stationary matmuls fault on HW):
    # top variants contract the first 32 K rows (j even), bottom the last 32;
    # replicated x2 down partitions for base-0 / base-64 slicing
    QYcT = np.tile(np.concatenate([QYc, Z32], axis=0), (2, 1))   # [128, 128]
    QYcB = np.tile(np.concatenate([Z32, QYc], axis=0), (2, 1))
    QYsT = np.tile(np.concatenate([QYs, Z32], axis=0), (2, 1))
    QYsB = np.tile(np.concatenate([Z32, QYs], axis=0), (2, 1))

    PRT = np.cos(2 * np.pi * np.outer(FRs, X) / 128)   # [r=64, X=128]
    PIT = np.sin(2 * np.pi * np.outer(FRs, X) / 128)
    PRT[32, :] = 0.0
    PIT[32, :] = 0.0
    # X-first synthesis const [128=(kxR,kxI), 256=(rg 2, X)]
    PPbig = np.zeros((128, 256))
    PPbig[0:64, 0:128] = PRT
    PPbig[64:128, 0:128] = -PIT
    PPbig[0:64, 128:256] = PIT
    PPbig[64:128, 128:256] = PRT

    # direct fr path: fr_i = Rx @ f_i @ Cy^T (pure 2x Fourier upsampling)
    ExRm = np.cos(2 * np.pi * np.outer(kx, x) / 64)
    ExIm = -np.sin(2 * np.pi * np.outer(kx, x) / 64)
    EyRm = np.cos(2 * np.pi * np.outer(c, y) / 64)
    EyIm = -np.sin(2 * np.pi * np.outer(c, y) / 64)
    QRm = s_q * w_c[None, :] * np.cos(2 * np.pi * np.outer(Y, c) / 128)
    QIm = s_q * w_c[None, :] * np.sin(2 * np.pi * np.outer(Y, c) / 128)
    Rx = PRT.T @ ExRm - PIT.T @ ExIm                 # [128, 64] (PRT.T == PR)
    Cy = QRm @ EyRm - QIm @ EyIm                     # [128, 64]
    RxT = Rx.T                                       # [x=64, X=128]
    CyT = np.concatenate([Cy.T, Cy.T], axis=0)       # [128, 128] doubled rows

    ImI = np.concatenate([np.eye(128), -np.eye(128)], axis=1)  # [128, 256]

    bf = lambda a: np.ascontiguousarray(a, dtype=ml_dtypes.bfloat16)
    f32 = lambda a: np.ascontiguousarray(a, dtype=np.float32)
    return dict(ExF=bf(ExF), EyCT=bf(EyCT), EyST=bf(EyST),
                S_sel=bf(S_sel), tTu=f32(tTu), tTv=f32(tTv),
                QYcT=bf(QYcT), QYcB=bf(QYcB), QYsT=bf(QYsT), QYsB=bf(QYsB),
                PPbig=bf(PPbig), RxT=bf(RxT), CyT=bf(CyT), ImI=bf(ImI))


def _rot90_kernel(k):
    # z[..., i, j] = k[..., (-j) mod n, i]
    y = np.swapaxes(k, -2, -1)
    return np.concatenate([y[..., :1], y[..., :0:-1]], axis=-1)


def _symmetric_kernel(k):
    k1 = k
    k2 = _rot90_kernel(k1)
    k3 = _rot90_kernel(k2)
    k4 = _rot90_kernel(k3)
    k5 = np.swapaxes(k1, -2, -1)
    k6 = _rot90_kernel(k5)
    k7 = _rot90_kernel(k6)
    k8 = _rot90_kernel(k7)
    return (k1 + k2 + k3 + k4 + k5 + k6 + k7 + k8) / 8.0


def _prep_k_all(kernel_np):
    """kernel [1,8,16,64,64] -> k_all [128, 2048] conv-layout packed (bf16)."""
    ksym = _symmetric_kernel(kernel_np.astype(np.float64))[0]   # [8,16,64,64]
    K = np.fft.rfft2(ksym).real                                  # [8,16,64,33]
    Kc = np.transpose(K[:, :, :, :32], (0, 1, 3, 2)).copy()      # [i,j,c,kx]
    Kc[:, :, :, 32] = 0.0                                        # kx nyquist
    k_all = np.zeros((128, 2048), dtype=np.float32)
    for i in range(8):
        h, im = i // 4, i % 4
        for j in range(16):
            k_all[im * 32:(im + 1) * 32, j * 128 + h * 64: j * 128 + h * 64 + 64] = Kc[i, j]
    return np.ascontiguousarray(k_all, dtype=ml_dtypes.bfloat16)


# ---------------------------------------------------------------------------
# device program
# ---------------------------------------------------------------------------

def _bcast(ap, n, axis_pos=1):
    """Insert a zero-step broadcast dim of size n into an AP (after partition dim)."""
    dims = list(ap.ap)
    dims.insert(axis_pos, [0, n])
    return AP(ap.tensor, ap.offset, dims)


def _view(ap, offset_elems, dims):
    """Raw AP view on the same tensor: explicit offset (elems) + [step, count] dims."""
    return AP(ap.tensor, ap.offset + offset_elems, dims)


def build_program(reps=1, ablate=(), gps_subs=False, gps_prod8=0, gps_conv=False,
                  sub_pool8=4, pe_sub8=0, **_unused):
    """ablate: subset of {'cross','synth','conv','dma'} to skip (profiling)."""
    nc = bacc.Bacc("TRN2", target_bir_lowering=False)
    consts = _host_consts()

    f_in = nc.dram_tensor("f_in", [B_PER_CORE, C1, 64, 64], F32, kind="ExternalInput")
    k_in = nc.dram_tensor("k_all", [128, 2048], BF16, kind="ExternalInput")
    # transposed output layout [b, X, ch, Y]; host returns .transpose(0,2,1,3) view
    out_sh = nc.dram_tensor("out_sh", [B_PER_CORE, 128, NCH_OUT, 128], F32,
                            kind="ExternalOutput")

    cdr = {name: nc.inline_tensor(arr, name=f"c_{name}") for name, arr in consts.items()}

    with tile.TileContext(nc) as tc:
        with (
            tc.tile_pool(name="cp", bufs=1) as cp,
            tc.tile_pool(name="fld", bufs=1) as fld,     # u_all/v_all/fr_all
            tc.tile_pool(name="wk", bufs=3) as wk,       # small working tiles
            tc.tile_pool(name="mw", bufs=2) as mwp,      # conv wide tiles
            tc.tile_pool(name="wp", bufs=3) as wp,       # cross product blocks
            tc.tile_pool(name="crp", bufs=3) as crp,     # cross output staging
            tc.tile_pool(name="pp", bufs=1, space="PSUM") as pp,
        ):
            # ---- load constants (stage1 deps first, spread over rings) ----
            cs = {}
            const_rings = [nc.scalar, nc.gpsimd]
            order = ['ExF', 'RxT', 'EyCT', 'EyST', 'S_sel', 'CyT',
                     'tTu', 'tTv', 'PPbig', 'QYcT', 'QYcB', 'QYsT', 'QYsB',
                     'ImI']
            for ci, name in enumerate(order):
                arr = consts[name]
                dt = BF16 if arr.dtype == ml_dtypes.bfloat16 else F32
                t = cp.tile(list(arr.shape), dt, tag=f"c_{name}", name=f"cs_{name}")
                const_rings[ci % 2].dma_start(out=t[:], in_=cdr[name][:])
                cs[name] = t
            k_sb = cp.tile([128, 2048], BF16, tag="k_sb")
            nc.gpsimd.dma_start(out=k_sb[:], in_=k_in[:])

            u_all = fld.tile([128, 16 * 256], BF16, tag="u_all")
            v_all = fld.tile([128, 16 * 256], BF16, tag="v_all")
            fr_all = fld.tile([128, 8 * 256], F32, tag="fr_all")

            dma_tick = [0]
            # weighted ring pattern: SP is otherwise idle, favor it
            ring_pats = {
                0: [nc.sync, nc.scalar, nc.sync, nc.gpsimd, nc.sync, nc.scalar],
                1: [nc.sync, nc.scalar, nc.gpsimd],
                2: [nc.sync, nc.scalar, nc.sync, nc.gpsimd],
                3: [nc.sync, nc.sync, nc.scalar, nc.sync, nc.sync, nc.gpsimd],
                4: [nc.sync, nc.gpsimd, nc.sync, nc.gpsimd, nc.sync, nc.scalar],
            }
            import os as _os
            out_rings = ring_pats[int(_os.environ.get("KRING", "0"))]

            def out_dma(out_ap, in_ap):
                eng = out_rings[dma_tick[0] % len(out_rings)]
                dma_tick[0] += 1
                eng.dma_start(out=out_ap, in_=in_ap)

            prod_tick = [0]

            def prod_eng():
                i = prod_tick[0] % 8
                prod_tick[0] += 1
                return nc.gpsimd if i < gps_prod8 else nc.vector

            sub_tick = [0]

            def sub_eng():
                i = sub_tick[0] % 8
                sub_tick[0] += 1
                return nc.gpsimd if i < sub_pool8 else nc.vector

            def emit_cross_block(gI, gJ, b):
                """cross products for channel groups gI x gJ, one sample.

                One fused [128, 2048] product op per W-block (16 pairs),
                one fused subtract + one 16-channel DMA per off-diag block."""
                W1 = wp.tile([128, 2048], BF16, tag="W1", name="W1")
                # W1[(a, bl, f)] = u[4gI+a] * v[4gJ+bl]
                in0 = _view(u_all[:], gI * 1024 + b * 128,
                            [u_all[:].ap[0], [256, 4], [0, 4], [1, 128]])
                in1 = _view(v_all[:], gJ * 1024 + b * 128,
                            [v_all[:].ap[0], [0, 4], [256, 4], [1, 128]])
                prod_eng().tensor_mul(
                    W1[:].rearrange("p (a c f) -> p a c f", a=4, c=4), in0, in1)
                if gI != gJ:
                    W2 = wp.tile([128, 2048], BF16, tag="W2", name="W2")
                    # W2[(bl, a, f)] = u[4gJ+bl] * v[4gI+a]
                    in0 = _view(u_all[:], gJ * 1024 + b * 128,
                                [u_all[:].ap[0], [256, 4], [0, 4], [1, 128]])
                    in1 = _view(v_all[:], gI * 1024 + b * 128,
                                [v_all[:].ap[0], [0, 4], [256, 4], [1, 128]])
                    prod_eng().tensor_mul(
                        W2[:].rearrange("p (c a f) -> p c a f", c=4, a=4), in0, in1)
                    # cr[(a, bl, f)] = W1[(a, bl, f)] - W2[(bl, a, f)]
                    cr = crp.tile([128, 2048], F32, tag="crb", name="crb")
                    st_i = sub_tick[0] % 8
                    sub_tick[0] += 1
                    if st_i < pe_sub8:
                        # subtract on the TensorEngine: psC = I@W1q - I@W2q',
                        # f32 chunk copies land on ACT
                        for q in range(4):
                            psC = pp.tile([128, 512], F32,
                                          tag=f"bankF{2 + q % 2}", bufs=1,
                                          name="psC")
                            w2q = _view(W2[:], q * 128,
                                        [W2[:].ap[0], [512, 4], [1, 128]])
                            nc.tensor.matmul(psC[:], cs["ImI"][:, 0:128],
                                             W1[:, q * 512:(q + 1) * 512],
                                             start=True, stop=False)
                            nc.tensor.matmul(
                                psC[:].rearrange("p (c f) -> p c f", c=4),
                                cs["ImI"][:, 128:256], w2q,
                                start=False, stop=True)
                            nc.scalar.copy(out=cr[:, q * 512:(q + 1) * 512],
                                           in_=psC[:])
                    else:
                        in1s = _view(W2[:], 0,
                                     [W2[:].ap[0], [128, 4], [512, 4], [1, 128]])
                        eng = nc.gpsimd if st_i < pe_sub8 + sub_pool8 else nc.vector
                        eng.tensor_sub(
                            cr[:].rearrange("p (a c f) -> p a c f", a=4, c=4),
                            W1[:].rearrange("p (a c f) -> p a c f", a=4, c=4), in1s)
                    # pair channels are contiguous per a only (stride 15-a
                    # between a rows) -> one 4-channel DMA per a
                    if 'dma' not in ablate:
                        for ai in range(4):
                            pch = 8 + _PAIR_IDX[(4 * gI + ai, 4 * gJ)]
                            out_dma(out_sh[b, :, pch:pch + 4, :],
                                    cr[:, ai * 512:(ai + 1) * 512].rearrange(
                                        "x (c y) -> x c y", c=4))
                else:
                    for ai in range(3):
                        a = 4 * gI + ai
                        cnt = 3 - ai
                        cr = crp.tile([128, 512], F32, tag="cr", name="cr")
                        in0 = _view(W1[:], ai * 512 + (ai + 1) * 128,
                                    [W1[:].ap[0], [128, cnt], [1, 128]])
                        in1 = _view(W1[:], (ai + 1) * 512 + ai * 128,
                                    [W1[:].ap[0], [512, cnt], [1, 128]])
                        sub_eng().tensor_sub(
                            cr[:, 0:cnt * 128].rearrange(
                                "p (cb f) -> p cb f", cb=cnt), in0, in1)
                        pch = 8 + _PAIR_IDX[(a, a + 1)]
                        if 'dma' not in ablate:
                            # small diag transfers: pin to the cheap SP ring
                            nc.sync.dma_start(
                                out=out_sh[b, :, pch:pch + cnt, :],
                                in_=cr[:, 0:cnt * 128].rearrange(
                                    "x (c y) -> x c y", c=cnt))

            def emit_stage1(b, st):
                A_ch = []
                T1s = []
                for ip in range(4):
                    fsb = wk.tile([64, 128], F32, tag="fsb", name="fsb")
                    nc.sync.dma_start(
                        out=fsb[:].rearrange("x (i y) -> x i y", i=2),
                        in_=f_in[b, 2 * ip:2 * ip + 2].rearrange("i x y -> x i y"))
                    fsb_bt = wk.tile([64, 128], BF16, tag="fsbb", name="fsbb")
                    nc.vector.tensor_copy(fsb_bt[:], fsb[:])
                    fsb_b = fsb_bt[:]
                    psA = pp.tile([128, 192], F32, tag="bankA", bufs=2, name="psA")
                    nc.tensor.matmul(psA[:], fsb_b, cs["ExF"][:], start=True, stop=True)
                    for iloc in range(2):
                        a_t = wk.tile([64, 192], BF16, tag=f"ach{2*ip+iloc}",
                                      name=f"ach{2*ip+iloc}")
                        nc.vector.tensor_copy(a_t[:], psA[iloc * 64:(iloc + 1) * 64, :])
                        A_ch.append(a_t)
                    # fr path: T1 = [f_i^T Rx^T | f_{i+1}^T Rx^T]  ([y, X] per channel)
                    psT1 = pp.tile([128, 128], F32, tag="bankA", bufs=2, name="psT1")
                    nc.tensor.matmul(psT1[:], fsb_b, cs["RxT"][:], start=True, stop=True)
                    t1sb = wk.tile([128, 128], BF16, tag=f"t1sb{ip}", name=f"t1sb{ip}")
                    if b == 0:
                        nc.vector.tensor_copy(t1sb[:], psT1[:])
                    else:
                        nc.scalar.copy(out=t1sb[:], in_=psT1[:])
                    T1s.append(t1sb)
                st['A_ch'] = A_ch
                st['T1s'] = T1s

            def emit_stage2(b, st):
                A_ch = st['A_ch']
                # out free = [F_R(kx64) | F_I(kx64)] per tile
                psFcv = [pp.tile([128, 128], F32, tag=f"bankF{2+h}", name=f"psFcv{h}")
                         for h in range(2)]
                EyC, EyS = cs["EyCT"], cs["EyST"]
                for i in range(8):
                    A_RI = A_ch[i][:, 0:128]     # [A_R | A_I]
                    A_IS = A_ch[i][:, 64:192]    # [A_I | -A_R]
                    h, im = i // 4, i % 4
                    sl = slice(im * 32, (im + 1) * 32)
                    tp = (0, im * 32)
                    nc.tensor.matmul(psFcv[h][sl, :], EyC[0:64, :], A_RI,
                                     start=True, stop=False, tile_position=tp)
                    nc.tensor.matmul(psFcv[h][sl, :], EyS[0:64, :], A_IS,
                                     start=False, stop=True, tile_position=tp)

                Fcv = wk.tile([128, 256], BF16, tag="Fcv", name="Fcv")
                for h in range(2):
                    if b == 0:
                        nc.vector.tensor_copy(Fcv[:, h * 64:(h + 1) * 64],
                                              psFcv[h][:, 0:64])
                        nc.vector.tensor_copy(Fcv[:, 128 + h * 64:128 + (h + 1) * 64],
                                              psFcv[h][:, 64:128])
                    else:
                        nc.scalar.copy(out=Fcv[:, h * 64:(h + 1) * 64],
                                       in_=psFcv[h][:, 0:64])
                        nc.scalar.copy(out=Fcv[:, 128 + h * 64:128 + (h + 1) * 64],
                                       in_=psFcv[h][:, 64:128])
                st['Fcv'] = Fcv

            def emit_conv(b, st):
                Fcv = st['Fcv']
                Mw = []
                for RI in range(2):
                    m_t = mwp.tile([128, 2048], BF16, tag=f"mw{RI}", name=f"mw{RI}")
                    in0 = _bcast(Fcv[:, RI * 128:(RI + 1) * 128], 16)
                    conv_eng = nc.gpsimd if (gps_conv or b == 0) else nc.vector
                    conv_eng.tensor_mul(
                        m_t[:].rearrange("p (j f) -> p j f", j=16),
                        in0,
                        k_sb[:].rearrange("p (j f) -> p j f", j=16))
                    Mw.append(m_t)

                # selector-transpose: psT2 rows 0-63 = acv_I^T [kx, (j,c)],
                # rows 64-127 = acv_R^T (i-sum via stationary=Mw, moving=S_sel)
                psT2 = pp.tile([128, 512], F32, tag="bankT", bufs=1, name="psT2")
                for RI in range(2):
                    rows = slice(64, 128) if RI == 0 else slice(0, 64)
                    for j in range(16):
                        for h in range(2):
                            lhsT = _view(Mw[RI][:], j * 128 + h * 64,
                                         [Mw[RI][:].ap[0], [1, 64]])
                            nc.tensor.matmul(
                                psT2[rows, j * 32:(j + 1) * 32],
                                lhsT, cs["S_sel"][:],
                                start=(h == 0), stop=(h == 1))
                # uncurl in transposed layout: BT [128=(kxR,kxI), 512=(j,c)]
                BTu = wk.tile([128, 512], BF16, tag="BTu", name="BTu")
                BTv = wk.tile([128, 512], BF16, tag="BTv", name="BTv")
                nc.vector.tensor_mul(BTu[:], psT2[:], cs["tTu"][:])
                nc.vector.tensor_mul(BTv[:], psT2[:], cs["tTv"][:])
                st['BT'] = (BTu, BTv)

            def emit_fr(b, st):
                # fr direct: fr_i = (T1_i)^T @ Cy^T via one matmul per channel
                for i in range(8):
                    ip, iloc = i // 2, i % 2
                    t1 = st['T1s'][ip][iloc * 64:(iloc + 1) * 64, :]
                    psUf = pp.tile([128, 128], F32, tag="bankU", bufs=1, name="psUf")
                    nc.tensor.matmul(psUf[:], t1,
                                     cs["CyT"][iloc * 64:(iloc + 1) * 64, :],
                                     start=True, stop=True)
                    nc.scalar.copy(out=fr_all[:, i * 256 + b * 128:i * 256 + (b + 1) * 128],
                                   in_=psUf[:])
                if 'dma' not in ablate:
                    frv = _view(fr_all[:], b * 128,
                                [fr_all[:].ap[0], [256, 8], [1, 128]])
                    nc.sync.dma_start(out=out_sh[b, :, 0:8, :], in_=frv)

            uvcp_tick = [0]

            def emit_synth_group(b, st, g):
                """synthesize u and v channels 4g..4g+3 via X-first 2-stage DFT."""
                BTu, BTv = st['BT']
                for fi, (BT, dest) in enumerate(((BTu, u_all), (BTv, v_all))):
                    # X-stage: psHT [(4j, c), (rg, X)] = BT-slice^T @ PPbig
                    psHT = pp.tile([128, 256], F32, tag=f"bankF{fi}", name="psHT")
                    nc.tensor.matmul(psHT[:], BT[:, g * 128:(g + 1) * 128],
                                     cs["PPbig"][:], start=True, stop=True)
                    # PE stationary base partition must be 0/32/64: split rows
                    HTa = wk.tile([64, 256], BF16, tag=f"HTa{fi}", name=f"HTa{fi}")
                    HTb = wk.tile([64, 256], BF16, tag=f"HTb{fi}", name=f"HTb{fi}")
                    nc.scalar.copy(out=HTa[:], in_=psHT[0:64, :])
                    nc.scalar.copy(out=HTb[:], in_=psHT[64:128, :])
                    # Y-stage: psF[X, (4j, Y)] = sum_c H_R QYc + H_I QYs
                    # (K=64 zero-padded: K=32 stationary matmuls fault on HW)
                    psF = pp.tile([128, 512], F32, tag=f"bankF{2 + fi}", name="psF")
                    for jl in range(4):
                        HT = HTa if jl < 2 else HTb
                        qc = cs["QYcT"] if jl % 2 == 0 else cs["QYcB"]
                        qs = cs["QYsT"] if jl % 2 == 0 else cs["QYsB"]
                        osl = psF[:, jl * 128:(jl + 1) * 128]
                        nc.tensor.matmul(osl, HT[:, 0:128], qc[0:64, :],
                                         start=True, stop=False)
                        nc.tensor.matmul(osl, HT[:, 128:256], qs[0:64, :],
                                         start=False, stop=True)
                    dsl = _view(dest[:], (4 * g) * 256 + b * 128,
                                [dest[:].ap[0], [256, 4], [1, 128]])
                    nc.scalar.copy(out=dsl, in_=psF[:].rearrange(
                        "p (c y) -> p c y", c=4))
                    uvcp_tick[0] += 1

            for rep in range(reps):
                st = {b: {} for b in range(B_PER_CORE)}
                for b in range(B_PER_CORE):
                    emit_stage1(b, st[b])
                    emit_stage2(b, st[b])
                    if 'conv' in ablate:
                        continue
                    emit_conv(b, st[b])
                if 'conv' in ablate:
                    continue
                if 'synth' in ablate:
                    continue
                for b in range(B_PER_CORE):
                    emit_fr(b, st[b])
                for g in range(4):
                    for b in range(B_PER_CORE):
                        emit_synth_group(b, st[b], g)
                        if 'cross' in ablate:
                            continue
                        for gI in range(g + 1):
                            emit_cross_block(gI, g, b)
    nc.compile()
    return nc


# ---------------------------------------------------------------------------
# entry point
# ---------------------------------------------------------------------------

_PROGRAM = {}


def _get_program(reps=1, ablate=(), **kw):
    global _PROGRAM
    import os
    if 'gps_subs' not in kw:
        kw['gps_subs'] = os.environ.get("KGPS", "0") == "1"
    if 'gps_prod8' not in kw:
        kw['gps_prod8'] = int(os.environ.get("KGPSP", "0"))
    if 'gps_conv' not in kw:
        kw['gps_conv'] = os.environ.get("KGPSC", "1") == "1"
    if 'sub_pool8' not in kw:
        kw['sub_pool8'] = int(os.environ.get("KSUBP", "8"))
    if 'pe_sub8' not in kw:
        kw['pe_sub8'] = int(os.environ.get("KPESUB", "0"))
    key = (reps, tuple(sorted(ablate)), tuple(sorted(kw.items())))
    if key not in _PROGRAM:
        _PROGRAM[key] = build_program(reps, ablate=ablate, **kw)
    return _PROGRAM[key]


LAST_EXEC_NS = None
LAST_RESULT = None


def kernel(f, kernel):
    global LAST_EXEC_NS, LAST_RESULT
    f = np.ascontiguousarray(f, dtype=np.float32)
    k_all = _prep_k_all(np.asarray(kernel))
    nc = _get_program()
    in_maps = [
        {"f_in": f[2 * c:2 * c + 2], "k_all": k_all} for c in range(N_CORES)
    ]
    import os
    trace = bool(os.environ.get("KERNEL_TRACE"))
    res = run_bass_kernel_spmd(nc, in_maps, list(range(N_CORES)), trace=trace)
    LAST_RESULT = res
    if res.exec_time_ns is not None:
        LAST_EXEC_NS = res.exec_time_ns
    out = np.concatenate([res.results[c]["out_sh"] for c in range(N_CORES)], axis=0)
    # device layout is [b, X, ch, Y]; return the [b, ch, X, Y] view
    return out.transpose(0, 2, 1, 3)


# revision 105
# speedup vs baseline: 1.0598x; 1.0112x over previous
"""Trainium2 Bass kernel for nn_EquivariantLayer (spectral equivariant layer).

Strategy (data-parallel over batch, 2 samples/core x 8 cores):
  All FFTs are expressed as real DFT matmuls on the TensorEngine with layouts
  chosen so no corner-turn transposes are ever needed:

    stage1:  A = f^T @ [ExR^T | ExI^T]          (contract x; out [y, (RI,kx)])
    stage2:  F = Ey @ A                          (contract y; out [c, kx], c-major)
    conv:    M = F (*) K elementwise (K = rfft2(sym kernel) is REAL since the
             symmetrized kernel is D4-symmetric); the i-reduction runs as a
             "selector-transpose" matmul (stationary = M-slice, moving =
             selector) so the conv spectrum lands TRANSPOSED [kx, (j, c)]
    uncurl:  TO_U = i*t, TO_V = i*s are pure-imaginary -> one fused real mult
             per field over the partition-stacked [acv_I; acv_R] PSUM tile
    synth:   X-first 2-stage iDFT: psHT[(4j,c),(RI,X)] = BT^T @ PPbig, then
             psF[X,(4j,Y)] = sum_c H_R QYc + H_I QYs with K=64 zero-padded
             stationaries (K=32 stationary matmuls fault on TRN2 HW)
    cross:   u_a v_b - u_b v_a: fused [128,2048] bf16 products (16 pairs per
             DVE op via zero-step broadcast APs), f32 subtract on gpsimd/DVE

  The compute pipeline runs in bf16 (PE at 1 cycle/row, DVE 2x mode); PSUM
  accumulation stays fp32 and the final subtraction materializes fp32 output.
  Output DMAs round-robin over the SP / ACT / SWDGE rings (SP-weighted).
  HW constraints honored: gpsimd never touches PSUM; tensor_tensor operands
  share a partition base; PE stationary bases are 0/32/64 with K >= 64.

Output [16, 128, 128, 128] f32 (~134 MB) dominates traffic (memory regime).
"""
import sys
import numpy as np
import ml_dtypes

if '/opt/trn_rl_repo' not in sys.path:
    sys.path.insert(0, '/opt/trn_rl_repo')

import concourse.bass as bass
from concourse import bacc
import concourse.mybir as mybir
import concourse.tile as tile
from concourse.bass import AP
from concourse.bass_utils import run_bass_kernel_spmd

F32 = mybir.dt.float32
BF16 = mybir.dt.bfloat16
N_CORES = 8
B_PER_CORE = 2
C1, C2, N1, N2 = 8, 16, 64, 128
NCH_OUT = 128  # 8 fr + 120 cross

I_IDX, J_IDX = np.triu_indices(C2, 1)
_PAIR_IDX = {}
for _p, (_a, _b) in enumerate(zip(I_IDX, J_IDX)):
    _PAIR_IDX[(int(_a), int(_b))] = _p


# ---------------------------------------------------------------------------
# host-side constant construction
# ---------------------------------------------------------------------------

def _host_consts():
    x = np.arange(64)
    kx = np.arange(64)
    c = np.arange(32)
    y = np.arange(64)
    X = np.arange(128)
    Y = np.arange(128)

    FRs = np.where(kx <= 32, kx, kx - 64).astype(np.float64)  # signed row freq

    ExR = np.cos(2 * np.pi * np.outer(kx, x) / 64)   # [kx, x]
    ExI = -np.sin(2 * np.pi * np.outer(kx, x) / 64)
    # [A_R | A_I | -A_R] so stage2 fuses R/I into two matmuls
    ExF = np.concatenate([ExR.T, ExI.T, -ExR.T], axis=1)   # [x, 192]

    # F_R = C A_R + S A_I ; F_I = C A_I + S (-A_R)   (C=cos, S=sin)
    # replicated x2 down partitions so base-0 and base-64 slices both exist
    EyCT = np.tile(np.cos(2 * np.pi * np.outer(c, y) / 64).T, (2, 1))  # [128, 32]
    EyST = np.tile(np.sin(2 * np.pi * np.outer(c, y) / 64).T, (2, 1))

    S_sel = np.zeros((128, 32))
    for im in range(4):
        S_sel[im * 32 + np.arange(32), np.arange(32)] = 1.0

    den = FRs[None, :] ** 2 + c[:, None].astype(np.float64) ** 2
    den[0, 0] = 1.0
    t_u = c[:, None] / den                           # [32, 64]
    s_v = -FRs[None, :] / den

    # uncurl consts in transposed [kx, (j-rep 16, c 32)] layout, partition-
    # aligned with psT2 = [acv_I (rows 0-63); acv_R (rows 64-127)]:
    #   BTu = psT2 * [-t; +t],  BTv = psT2 * [-s; +s]
    tmat = np.tile(t_u.T[:, None, :], (1, 16, 1)).reshape(64, 512)
    smat = np.tile(s_v.T[:, None, :], (1, 16, 1)).reshape(64, 512)
    tTu = np.concatenate([-tmat, tmat], axis=0)      # [128, 512]
    tTv = np.concatenate([-smat, smat], axis=0)

    w_c = np.where(c == 0, 1.0, 2.0)
    s_q = 2.0 / (128.0 * 128.0)
    QYc = s_q * w_c[:, None] * np.cos(2 * np.pi * np.outer(c, Y) / 128)   # [32, 128]
    QYs = -s_q * w_c[:, None] * np.sin(2 * np.pi * np.outer(c, Y) / 128)
    Z32 = np.zeros((32, 128))
    # K=64 zero-padded Y-stage consts (K=32 stationary matmuls fault on HW):
    # top variants contract the first 32 K rows (j even), bottom the last 32;
    # replicated x2 down partitions for base-0 / base-64 slicing
    QYcT = np.tile(np.concatenate([QYc, Z32], axis=0), (2, 1))   # [128, 128]
    QYcB = np.tile(np.concatenate([Z32, QYc], axis=0), (2, 1))
    QYsT = np.tile(np.concatenate([QYs, Z32], axis=0), (2, 1))
    QYsB = np.tile(np.concatenate([Z32, QYs], axis=0), (2, 1))

    PRT = np.cos(2 * np.pi * np.outer(FRs, X) / 128)   # [r=64, X=128]
    PIT = np.sin(2 * np.pi * np.outer(FRs, X) / 128)
    PRT[32, :] = 0.0
    PIT[32, :] = 0.0
    # X-first synthesis const [128=(kxR,kxI), 256=(rg 2, X)]
    PPbig = np.zeros((128, 256))
    PPbig[0:64, 0:128] = PRT
    PPbig[64:128, 0:128] = -PIT
    PPbig[0:64, 128:256] = PIT
    PPbig[64:128, 128:256] = PRT

    # direct fr path: fr_i = Rx @ f_i @ Cy^T (pure 2x Fourier upsampling)
    ExRm = np.cos(2 * np.pi * np.outer(kx, x) / 64)
    ExIm = -np.sin(2 * np.pi * np.outer(kx, x) / 64)
    EyRm = np.cos(2 * np.pi * np.outer(c, y) / 64)
    EyIm = -np.sin(2 * np.pi * np.outer(c, y) / 64)
    QRm = s_q * w_c[None, :] * np.cos(2 * np.pi * np.outer(Y, c) / 128)
    QIm = s_q * w_c[None, :] * np.sin(2 * np.pi * np.outer(Y, c) / 128)
    Rx = PRT.T @ ExRm - PIT.T @ ExIm                 # [128, 64] (PRT.T == PR)
    Cy = QRm @ EyRm - QIm @ EyIm                     # [128, 64]
    RxT = Rx.T                                       # [x=64, X=128]
    CyT = np.concatenate([Cy.T, Cy.T], axis=0)       # [128, 128] doubled rows

    ImI = np.concatenate([np.eye(128), -np.eye(128)], axis=1)  # [128, 256]

    bf = lambda a: np.ascontiguousarray(a, dtype=ml_dtypes.bfloat16)
    f32 = lambda a: np.ascontiguousarray(a, dtype=np.float32)
    return dict(ExF=bf(ExF), EyCT=bf(EyCT), EyST=bf(EyST),
                S_sel=bf(S_sel), tTu=f32(tTu), tTv=f32(tTv),
                QYcT=bf(QYcT), QYcB=bf(QYcB), QYsT=bf(QYsT), QYsB=bf(QYsB),
                PPbig=bf(PPbig), RxT=bf(RxT), CyT=bf(CyT), ImI=bf(ImI))


def _rot90_kernel(k):
    # z[..., i, j] = k[..., (-j) mod n, i]
    y = np.swapaxes(k, -2, -1)
    return np.concatenate([y[..., :1], y[..., :0:-1]], axis=-1)


def _symmetric_kernel(k):
    k1 = k
    k2 = _rot90_kernel(k1)
    k3 = _rot90_kernel(k2)
    k4 = _rot90_kernel(k3)
    k5 = np.swapaxes(k1, -2, -1)
    k6 = _rot90_kernel(k5)
    k7 = _rot90_kernel(k6)
    k8 = _rot90_kernel(k7)
    return (k1 + k2 + k3 + k4 + k5 + k6 + k7 + k8) / 8.0


def _prep_k_all(kernel_np):
    """kernel [1,8,16,64,64] -> k_all [128, 2048] conv-layout packed (bf16)."""
    ksym = _symmetric_kernel(kernel_np.astype(np.float64))[0]   # [8,16,64,64]
    K = np.fft.rfft2(ksym).real                                  # [8,16,64,33]
    Kc = np.transpose(K[:, :, :, :32], (0, 1, 3, 2)).copy()      # [i,j,c,kx]
    Kc[:, :, :, 32] = 0.0                                        # kx nyquist
    k_all = np.zeros((128, 2048), dtype=np.float32)
    for i in range(8):
        h, im = i // 4, i % 4
        for j in range(16):
            k_all[im * 32:(im + 1) * 32, j * 128 + h * 64: j * 128 + h * 64 + 64] = Kc[i, j]
    return np.ascontiguousarray(k_all, dtype=ml_dtypes.bfloat16)


# ---------------------------------------------------------------------------
# device program
# ---------------------------------------------------------------------------

def _bcast(ap, n, axis_pos=1):
    """Insert a zero-step broadcast dim of size n into an AP (after partition dim)."""
    dims = list(ap.ap)
    dims.insert(axis_pos, [0, n])
    return AP(ap.tensor, ap.offset, dims)


def _view(ap, offset_elems, dims):
    """Raw AP view on the same tensor: explicit offset (elems) + [step, count] dims."""
    return AP(ap.tensor, ap.offset + offset_elems, dims)


def build_program(reps=1, ablate=(), gps_subs=False, gps_prod8=0, gps_conv=False,
                  sub_pool8=4, pe_sub8=0, **_unused):
    """ablate: subset of {'cross','synth','conv','dma'} to skip (profiling)."""
    nc = bacc.Bacc("TRN2", target_bir_lowering=False)
    consts = _host_consts()

    f_in = nc.dram_tensor("f_in", [B_PER_CORE, C1, 64, 64], F32, kind="ExternalInput")
    k_in = nc.dram_tensor("k_all", [128, 2048], BF16, kind="ExternalInput")
    # transposed output layout [b, X, ch, Y]; host returns .transpose(0,2,1,3) view
    out_sh = nc.dram_tensor("out_sh", [B_PER_CORE, 128, NCH_OUT, 128], F32,
                            kind="ExternalOutput")

    cdr = {name: nc.inline_tensor(arr, name=f"c_{name}") for name, arr in consts.items()}

    with tile.TileContext(nc) as tc:
        with (
            tc.tile_pool(name="cp", bufs=1) as cp,
            tc.tile_pool(name="fld", bufs=1) as fld,     # u_all/v_all/fr_all
            tc.tile_pool(name="wk", bufs=3) as wk,       # small working tiles
            tc.tile_pool(name="mw", bufs=2) as mwp,      # conv wide tiles
            tc.tile_pool(name="wp", bufs=3) as wp,       # cross product blocks
            tc.tile_pool(name="crp", bufs=3) as crp,     # cross output staging
            tc.tile_pool(name="pp", bufs=1, space="PSUM") as pp,
        ):
            # ---- load constants (stage1 deps first, spread over rings) ----
            cs = {}
            const_rings = [nc.scalar, nc.gpsimd]
            order = ['ExF', 'RxT', 'EyCT', 'EyST', 'S_sel', 'CyT',
                     'tTu', 'tTv', 'PPbig', 'QYcT', 'QYcB', 'QYsT', 'QYsB',
                     'ImI']
            for ci, name in enumerate(order):
                arr = consts[name]
                dt = BF16 if arr.dtype == ml_dtypes.bfloat16 else F32
                t = cp.tile(list(arr.shape), dt, tag=f"c_{name}", name=f"cs_{name}")
                const_rings[ci % 2].dma_start(out=t[:], in_=cdr[name][:])
                cs[name] = t
            k_sb = cp.tile([128, 2048], BF16, tag="k_sb")
            nc.gpsimd.dma_start(out=k_sb[:], in_=k_in[:])

            u_all = fld.tile([128, 16 * 256], BF16, tag="u_all")
            v_all = fld.tile([128, 16 * 256], BF16, tag="v_all")
            fr_all = fld.tile([128, 8 * 256], F32, tag="fr_all")

            dma_tick = [0]
            # weighted ring pattern: SP is otherwise idle, favor it
            ring_pats = {
                0: [nc.sync, nc.scalar, nc.sync, nc.gpsimd, nc.sync, nc.scalar],
                1: [nc.sync, nc.scalar, nc.gpsimd],
                2: [nc.sync, nc.scalar, nc.sync, nc.gpsimd],
                3: [nc.sync, nc.sync, nc.scalar, nc.sync, nc.sync, nc.gpsimd],
                4: [nc.sync, nc.gpsimd, nc.sync, nc.gpsimd, nc.sync, nc.scalar],
            }
            import os as _os
            out_rings = ring_pats[int(_os.environ.get("KRING", "0"))]

            def out_dma(out_ap, in_ap):
                eng = out_rings[dma_tick[0] % len(out_rings)]
                dma_tick[0] += 1
                eng.dma_start(out=out_ap, in_=in_ap)

            prod_tick = [0]

            def prod_eng():
                i = prod_tick[0] % 8
                prod_tick[0] += 1
                return nc.gpsimd if i < gps_prod8 else nc.vector

            sub_tick = [0]

            def sub_eng():
                i = sub_tick[0] % 8
                sub_tick[0] += 1
                return nc.gpsimd if i < sub_pool8 else nc.vector

            def emit_cross_block(gI, gJ, b):
                """cross products for channel groups gI x gJ, one sample.

                One fused [128, 2048] product op per W-block (16 pairs),
                one fused subtract + one 16-channel DMA per off-diag block."""
                W1 = wp.tile([128, 2048], BF16, tag="W1", name="W1")
                # W1[(a, bl, f)] = u[4gI+a] * v[4gJ+bl]
                in0 = _view(u_all[:], gI * 1024 + b * 128,
                            [u_all[:].ap[0], [256, 4], [0, 4], [1, 128]])
                in1 = _view(v_all[:], gJ * 1024 + b * 128,
                            [v_all[:].ap[0], [0, 4], [256, 4], [1, 128]])
                prod_eng().tensor_mul(
                    W1[:].rearrange("p (a c f) -> p a c f", a=4, c=4), in0, in1)
                if gI != gJ:
                    W2 = wp.tile([128, 2048], BF16, tag="W2", name="W2")
                    # W2[(bl, a, f)] = u[4gJ+bl] * v[4gI+a]
                    in0 = _view(u_all[:], gJ * 1024 + b * 128,
                                [u_all[:].ap[0], [256, 4], [0, 4], [1, 128]])
                    in1 = _view(v_all[:], gI * 1024 + b * 128,
                                [v_all[:].ap[0], [0, 4], [256, 4], [1, 128]])
                    prod_eng().tensor_mul(
                        W2[:].rearrange("p (c a f) -> p c a f", c=4, a=4), in0, in1)
                    # cr[(a, bl, f)] = W1[(a, bl, f)] - W2[(bl, a, f)]
                    cr = crp.tile([128, 2048], F32, tag="crb", name="crb")
                    st_i = sub_tick[0] % 8
                    sub_tick[0] += 1
                    if st_i < pe_sub8:
                        # subtract on the TensorEngine: psC = I@W1q - I@W2q',
                        # f32 chunk copies land on ACT
                        for q in range(4):
                            psC = pp.tile([128, 512], F32,
                                          tag=f"bankF{2 + q % 2}", bufs=1,
                                          name="psC")
                            w2q = _view(W2[:], q * 128,
                                        [W2[:].ap[0], [512, 4], [1, 128]])
                            nc.tensor.matmul(psC[:], cs["ImI"][:, 0:128],
                                             W1[:, q * 512:(q + 1) * 512],
                                             start=True, stop=False)
                            nc.tensor.matmul(
                                psC[:].rearrange("p (c f) -> p c f", c=4),
                                cs["ImI"][:, 128:256], w2q,
                                start=False, stop=True)
                            nc.scalar.copy(out=cr[:, q * 512:(q + 1) * 512],
                                           in_=psC[:])
                    else:
                        in1s = _view(W2[:], 0,
                                     [W2[:].ap[0], [128, 4], [512, 4], [1, 128]])
                        eng = nc.gpsimd if st_i < pe_sub8 + sub_pool8 else nc.vector
                        eng.tensor_sub(
                            cr[:].rearrange("p (a c f) -> p a c f", a=4, c=4),
                            W1[:].rearrange("p (a c f) -> p a c f", a=4, c=4), in1s)
                    # pair channels are contiguous per a only (stride 15-a
                    # between a rows) -> one 4-channel DMA per a
                    if 'dma' not in ablate:
                        for ai in range(4):
                            pch = 8 + _PAIR_IDX[(4 * gI + ai, 4 * gJ)]
                            out_dma(out_sh[b, :, pch:pch + 4, :],
                                    cr[:, ai * 512:(ai + 1) * 512].rearrange(
                                        "x (c y) -> x c y", c=4))
                else:
                    for ai in range(3):
                        a = 4 * gI + ai
                        cnt = 3 - ai
                        cr = crp.tile([128, 512], F32, tag="cr", name="cr")
                        in0 = _view(W1[:], ai * 512 + (ai + 1) * 128,
                                    [W1[:].ap[0], [128, cnt], [1, 128]])
                        in1 = _view(W1[:], (ai + 1) * 512 + ai * 128,
                                    [W1[:].ap[0], [512, cnt], [1, 128]])
                        sub_eng().tensor_sub(
                            cr[:, 0:cnt * 128].rearrange(
                                "p (cb f) -> p cb f", cb=cnt), in0, in1)
                        pch = 8 + _PAIR_IDX[(a, a + 1)]
                        if 'dma' not in ablate:
                            # small diag transfers: pin to the cheap SP ring,
                            # except the final block (SP is backlogged then)
                            deng = nc.scalar if gI == 3 else nc.sync
                            deng.dma_start(
                                out=out_sh[b, :, pch:pch + cnt, :],
                                in_=cr[:, 0:cnt * 128].rearrange(
                                    "x (c y) -> x c y", c=cnt))

            def emit_stage1(b, st):
                A_ch = []
                T1s = []
                for ip in range(4):
                    fsb = wk.tile([64, 128], F32, tag="fsb", name="fsb")
                    nc.sync.dma_start(
                        out=fsb[:].rearrange("x (i y) -> x i y", i=2),
                        in_=f_in[b, 2 * ip:2 * ip + 2].rearrange("i x y -> x i y"))
                    fsb_bt = wk.tile([64, 128], BF16, tag="fsbb", name="fsbb")
                    nc.vector.tensor_copy(fsb_bt[:], fsb[:])
                    fsb_b = fsb_bt[:]
                    psA = pp.tile([128, 192], F32, tag="bankA", bufs=2, name="psA")
                    nc.tensor.matmul(psA[:], fsb_b, cs["ExF"][:], start=True, stop=True)
                    for iloc in range(2):
                        a_t = wk.tile([64, 192], BF16, tag=f"ach{2*ip+iloc}",
                                      name=f"ach{2*ip+iloc}")
                        nc.vector.tensor_copy(a_t[:], psA[iloc * 64:(iloc + 1) * 64, :])
                        A_ch.append(a_t)
                    # fr path: T1 = [f_i^T Rx^T | f_{i+1}^T Rx^T]  ([y, X] per channel)
                    psT1 = pp.tile([128, 128], F32, tag="bankA", bufs=2, name="psT1")
                    nc.tensor.matmul(psT1[:], fsb_b, cs["RxT"][:], start=True, stop=True)
                    t1sb = wk.tile([128, 128], BF16, tag=f"t1sb{ip}", name=f"t1sb{ip}")
                    if b == 0:
                        nc.vector.tensor_copy(t1sb[:], psT1[:])
                    else:
                        nc.scalar.copy(out=t1sb[:], in_=psT1[:])
                    T1s.append(t1sb)
                st['A_ch'] = A_ch
                st['T1s'] = T1s

            def emit_stage2(b, st):
                A_ch = st['A_ch']
                # out free = [F_R(kx64) | F_I(kx64)] per tile
                psFcv = [pp.tile([128, 128], F32, tag=f"bankF{2+h}", name=f"psFcv{h}")
                         for h in range(2)]
                EyC, EyS = cs["EyCT"], cs["EyST"]
                for i in range(8):
                    A_RI = A_ch[i][:, 0:128]     # [A_R | A_I]
                    A_IS = A_ch[i][:, 64:192]    # [A_I | -A_R]
                    h, im = i // 4, i % 4
                    sl = slice(im * 32, (im + 1) * 32)
                    tp = (0, im * 32)
                    nc.tensor.matmul(psFcv[h][sl, :], EyC[0:64, :], A_RI,
                                     start=True, stop=False, tile_position=tp)
                    nc.tensor.matmul(psFcv[h][sl, :], EyS[0:64, :], A_IS,
                                     start=False, stop=True, tile_position=tp)

                Fcv = wk.tile([128, 256], BF16, tag="Fcv", name="Fcv")
                for h in range(2):
                    if b == 0:
                        nc.vector.tensor_copy(Fcv[:, h * 64:(h + 1) * 64],
                                              psFcv[h][:, 0:64])
                        nc.vector.tensor_copy(Fcv[:, 128 + h * 64:128 + (h + 1) * 64],
                                              psFcv[h][:, 64:128])
                    else:
                        nc.scalar.copy(out=Fcv[:, h * 64:(h + 1) * 64],
                                       in_=psFcv[h][:, 0:64])
                        nc.scalar.copy(out=Fcv[:, 128 + h * 64:128 + (h + 1) * 64],
                                       in_=psFcv[h][:, 64:128])
                st['Fcv'] = Fcv

            def emit_conv(b, st):
                Fcv = st['Fcv']
                Mw = []
                for RI in range(2):
                    m_t = mwp.tile([128, 2048], BF16, tag=f"mw{RI}", name=f"mw{RI}")
                    in0 = _bcast(Fcv[:, RI * 128:(RI + 1) * 128], 16)
                    conv_eng = nc.gpsimd if (gps_conv or b == 0) else nc.vector
                    conv_eng.tensor_mul(
                        m_t[:].rearrange("p (j f) -> p j f", j=16),
                        in0,
                        k_sb[:].rearrange("p (j f) -> p j f", j=16))
                    Mw.append(m_t)

                # selector-transpose: psT2 rows 0-63 = acv_I^T [kx, (j,c)],
                # rows 64-127 = acv_R^T (i-sum via stationary=Mw, moving=S_sel)
                psT2 = pp.tile([128, 512], F32, tag="bankT", bufs=1, name="psT2")
                for RI in range(2):
                    rows = slice(64, 128) if RI == 0 else slice(0, 64)
                    for j in range(16):
                        for h in range(2):
                            lhsT = _view(Mw[RI][:], j * 128 + h * 64,
                                         [Mw[RI][:].ap[0], [1, 64]])
                            nc.tensor.matmul(
                                psT2[rows, j * 32:(j + 1) * 32],
                                lhsT, cs["S_sel"][:],
                                start=(h == 0), stop=(h == 1))
                # uncurl in transposed layout: BT [128=(kxR,kxI), 512=(j,c)]
                BTu = wk.tile([128, 512], BF16, tag="BTu", name="BTu")
                BTv = wk.tile([128, 512], BF16, tag="BTv", name="BTv")
                nc.vector.tensor_mul(BTu[:], psT2[:], cs["tTu"][:])
                nc.vector.tensor_mul(BTv[:], psT2[:], cs["tTv"][:])
                st['BT'] = (BTu, BTv)

            def emit_fr(b, st):
                # fr direct: fr_i = (T1_i)^T @ Cy^T via one matmul per channel
                for i in range(8):
                    ip, iloc = i // 2, i % 2
                    t1 = st['T1s'][ip][iloc * 64:(iloc + 1) * 64, :]
                    psUf = pp.tile([128, 128], F32, tag="bankU", bufs=1, name="psUf")
                    nc.tensor.matmul(psUf[:], t1,
                                     cs["CyT"][iloc * 64:(iloc + 1) * 64, :],
                                     start=True, stop=True)
                    nc.scalar.copy(out=fr_all[:, i * 256 + b * 128:i * 256 + (b + 1) * 128],
                                   in_=psUf[:])
                if 'dma' not in ablate:
                    frv = _view(fr_all[:], b * 128,
                                [fr_all[:].ap[0], [256, 8], [1, 128]])
                    nc.sync.dma_start(out=out_sh[b, :, 0:8, :], in_=frv)

            uvcp_tick = [0]

            def emit_synth_group(b, st, g):
                """synthesize u and v channels 4g..4g+3 via X-first 2-stage DFT."""
                BTu, BTv = st['BT']
                for fi, (BT, dest) in enumerate(((BTu, u_all), (BTv, v_all))):
                    # X-stage: psHT [(4j, c), (rg, X)] = BT-slice^T @ PPbig
                    psHT = pp.tile([128, 256], F32, tag=f"bankF{fi}", name="psHT")
                    nc.tensor.matmul(psHT[:], BT[:, g * 128:(g + 1) * 128],
                                     cs["PPbig"][:], start=True, stop=True)
                    # PE stationary base partition must be 0/32/64: split rows
                    HTa = wk.tile([64, 256], BF16, tag=f"HTa{fi}", name=f"HTa{fi}")
                    HTb = wk.tile([64, 256], BF16, tag=f"HTb{fi}", name=f"HTb{fi}")
                    nc.scalar.copy(out=HTa[:], in_=psHT[0:64, :])
                    nc.scalar.copy(out=HTb[:], in_=psHT[64:128, :])
                    # Y-stage: psF[X, (4j, Y)] = sum_c H_R QYc + H_I QYs
                    # (K=64 zero-padded: K=32 stationary matmuls fault on HW)
                    psF = pp.tile([128, 512], F32, tag=f"bankF{2 + fi}", name="psF")
                    for jl in range(4):
                        HT = HTa if jl < 2 else HTb
                        qc = cs["QYcT"] if jl % 2 == 0 else cs["QYcB"]
                        qs = cs["QYsT"] if jl % 2 == 0 else cs["QYsB"]
                        osl = psF[:, jl * 128:(jl + 1) * 128]
                        nc.tensor.matmul(osl, HT[:, 0:128], qc[0:64, :],
                                         start=True, stop=False)
                        nc.tensor.matmul(osl, HT[:, 128:256], qs[0:64, :],
                                         start=False, stop=True)
                    dsl = _view(dest[:], (4 * g) * 256 + b * 128,
                                [dest[:].ap[0], [256, 4], [1, 128]])
                    nc.scalar.copy(out=dsl, in_=psF[:].rearrange(
                        "p (c y) -> p c y", c=4))
                    uvcp_tick[0] += 1

            for rep in range(reps):
                st = {b: {} for b in range(B_PER_CORE)}
                for b in range(B_PER_CORE):
                    emit_stage1(b, st[b])
                    emit_stage2(b, st[b])
                    if 'conv' in ablate:
                        continue
                    emit_conv(b, st[b])
                if 'conv' in ablate:
                    continue
                if 'synth' in ablate:
                    continue
                for b in range(B_PER_CORE):
                    emit_fr(b, st[b])
                for g in range(4):
                    for b in range(B_PER_CORE):
                        emit_synth_group(b, st[b], g)
                        if 'cross' in ablate:
                            continue
                        for gI in range(g + 1):
                            emit_cross_block(gI, g, b)
    nc.compile()
    return nc


# ---------------------------------------------------------------------------
# entry point
# ---------------------------------------------------------------------------

_PROGRAM = {}


def _get_program(reps=1, ablate=(), **kw):
    global _PROGRAM
    import os
    if 'gps_subs' not in kw:
        kw['gps_subs'] = os.environ.get("KGPS", "0") == "1"
    if 'gps_prod8' not in kw:
        kw['gps_prod8'] = int(os.environ.get("KGPSP", "0"))
    if 'gps_conv' not in kw:
        kw['gps_conv'] = os.environ.get("KGPSC", "1") == "1"
    if 'sub_pool8' not in kw:
        kw['sub_pool8'] = int(os.environ.get("KSUBP", "8"))
    if 'pe_sub8' not in kw:
        kw['pe_sub8'] = int(os.environ.get("KPESUB", "0"))
    key = (reps, tuple(sorted(ablate)), tuple(sorted(kw.items())))
    if key not in _PROGRAM:
        _PROGRAM[key] = build_program(reps, ablate=ablate, **kw)
    return _PROGRAM[key]


LAST_EXEC_NS = None
LAST_RESULT = None


def kernel(f, kernel):
    global LAST_EXEC_NS, LAST_RESULT
    f = np.ascontiguousarray(f, dtype=np.float32)
    k_all = _prep_k_all(np.asarray(kernel))
    nc = _get_program()
    in_maps = [
        {"f_in": f[2 * c:2 * c + 2], "k_all": k_all} for c in range(N_CORES)
    ]
    import os
    trace = bool(os.environ.get("KERNEL_TRACE"))
    res = run_bass_kernel_spmd(nc, in_maps, list(range(N_CORES)), trace=trace)
    LAST_RESULT = res
    if res.exec_time_ns is not None:
        LAST_EXEC_NS = res.exec_time_ns
    out = np.concatenate([res.results[c]["out_sh"] for c in range(N_CORES)], axis=0)
    # device layout is [b, X, ch, Y]; return the [b, ch, X, Y] view
    return out.transpose(0, 2, 1, 3)
